# revision 1
# baseline (speedup 1.0000x reference)
"""Trainium2 Bass kernel for nn_GCNN_87668872446200.

Two GCNConv+pool protein branches + two masif conv branches + dense head,
distributed over 8 NeuronCores as 4 feature-slices x 2 dest-node halves.

Per core:
  - xw = x @ W[:, fslice]  (PE matmul from host-pretransposed xT, bf16)
  - xw written to HBM; dma_gather pulls source rows for this core's edge half
    (512B descriptors, full DMA rate)
  - scatter-add realized as PE matmuls: S[128 edges, 64 dests] (host-built,
    norm-scaled, bf16, streamed from HBM) x gathered[128 edges, 256 feats]
    accumulated in PSUM per 64-dest block
  - +bias (DVE), LeakyReLU (ACT) -> h block [64, 256] bf16
  - per-graph mean-pool as PE matmul with host-built Mpool (folds 1/cnt)
  - masif branch: 4 graphs/core, reduces+ACT+tiny matmuls
  - one AllReduce (~272KB) merges pooled features + masif outputs
  - replicated dense head -> sigmoid -> [1, 32] output (core 0's is used)

All 8 cores run ONE identical program; per-core variation is in input data
(weight slices, gather indices, S/Mpool matrices, masks).
"""
import math
import numpy as np

# ---------------------------------------------------------------- constants
N_CORES = 8
N_FSLICE = 2      # feature slices (F // N_FSLICE features per core)
N_DPART = 4       # destination-node partitions
P = 128
BLK = 64          # dest nodes per scatter block (S width)
GRP = 8           # chunks per gather/S group (1024 idxs per dma_gather;
                  # HW fails somewhere in (1024, 2048] idxs per call)

# problem sizes (hardcoded per spec)
N_NODES, N_EDGES, F_DIM, B_GRAPHS, L_MAS, C_MAS = 10000, 80000, 1024, 32, 800, 16


class _Cfg:
    def __init__(self, n=N_NODES, e=N_EDGES, f=F_DIM, b=B_GRAPHS,
                 l=L_MAS, c=C_MAS):
        assert f % 512 == 0 and b == 32 and l % 80 == 0 and c % 2 == 0
        self.N, self.E, self.F, self.B, self.L, self.C = n, e, f, b, l, c
        self.NPAD = ((n + 511) // 512) * 512
        while (self.NPAD // N_DPART) % BLK or (self.NPAD % 512):
            self.NPAD += 512
        self.HALF = self.NPAD // N_DPART       # nodes per dest partition
        self.NBLK = self.HALF // BLK           # blocks per dest partition
        self.FSL = f // N_FSLICE               # features per core slice
        self.KC = f // P                       # k-chunks of contraction
        self.GPB = b // N_CORES                # graphs per core for masif
        self.LW = l // 80                      # avg-pool window (10)
        self.LB = 8                            # l-blocks for masif layout
        self.LBS = l // self.LB                # l-block size (100)
        assert self.LBS % self.LW == 0
        self.WPB = self.LBS // self.LW         # windows per l-block (10)


# ---------------------------------------------------------------- host prep
def _edge_plan(cfg, edge_index):
    """Per-half sorted/chunked scatter plans with a shared per-block chunk
    schedule (max over halves), padded to a multiple of GRP chunks."""
    row = np.asarray(edge_index[0]).astype(np.int64)
    col = np.asarray(edge_index[1]).astype(np.int64)
    loops = np.arange(cfg.N, dtype=np.int64)
    rows = np.concatenate([row, loops])
    cols = np.concatenate([col, loops])
    deg = np.bincount(cols, minlength=cfg.N).astype(np.float64)
    dinv = 1.0 / np.sqrt(deg)
    norm = (dinv[rows] * dinv[cols]).astype(np.float32)

    halves = []
    counts = np.zeros((N_DPART, cfg.NBLK), np.int64)
    for hf in range(N_DPART):
        lo, hi = hf * cfg.HALF, (hf + 1) * cfg.HALF
        sel = (cols >= lo) & (cols < hi)
        r, c, w = rows[sel], cols[sel], norm[sel]
        order = np.argsort(c, kind='stable')
        r, c, w = r[order], c[order], w[order]
        blk = (c - lo) // BLK
        starts = np.searchsorted(blk, np.arange(cfg.NBLK), side='left')
        ends = np.searchsorted(blk, np.arange(cfg.NBLK), side='right')
        counts[hf] = np.maximum((ends - starts + 127) // 128, 1)
        halves.append((r, c - lo, w, starts, ends))

    kj = counts.max(0)                       # shared chunks per block
    c_total = int(kj.sum())
    c_pad = ((c_total + GRP - 1) // GRP) * GRP
    kj[-1] += c_pad - c_total                # tail dummies on last block

    # shared schedule: list of (block, start_flag, stop_flag)
    sched = []
    for j in range(cfg.NBLK):
        for k in range(kj[j]):
            sched.append((j, k == 0, k == kj[j] - 1))
    assert len(sched) == c_pad

    # per-part streams
    srcs_all, s_all = [], []
    for hf in range(N_DPART):
        r, cl, w, starts, ends = halves[hf]
        srcs = np.zeros((c_pad, P), np.int16)
        smat = np.zeros((c_pad, P, BLK), np.float32)
        i = 0
        for j in range(cfg.NBLK):
            s0, e0 = starts[j], ends[j]
            for k in range(kj[j]):
                cs = s0 + k * P
                ce = min(cs + P, e0)
                if ce > cs:
                    n = ce - cs
                    srcs[i, :n] = r[cs:ce]
                    smat[i, np.arange(n), cl[cs:ce] - j * BLK] = w[cs:ce]
                i += 1
        srcs_all.append(srcs)
        s_all.append(smat)
    return sched, c_pad, srcs_all, s_all


def _wrap_idxs(srcs):
    """[C, 128] int16 -> wrapped [128, C*8] (idx j at [j%16 + 16*rep, j//16])."""
    flat = srcs.reshape(-1)
    w = flat.reshape(-1, 16).T                # [16, C*8]
    return np.tile(w, (8, 1)).astype(np.int16)


def _group_s(smat, dt):
    """[C, 128, 64] -> [C//GRP, 128, GRP*64] grouped for contiguous loads."""
    c = smat.shape[0]
    g = smat.reshape(c // GRP, GRP, P, BLK).transpose(0, 2, 1, 3)
    return np.ascontiguousarray(g.reshape(c // GRP, P, GRP * BLK)).astype(dt)


def _mpool(cfg, batch, hf, dt):
    """[HALF, B] matrix folding 1/cnt, zero rows for pad nodes."""
    batch = np.asarray(batch).astype(np.int64)
    cnt = np.bincount(batch, minlength=cfg.B).astype(np.float64)
    cinv = 1.0 / np.maximum(cnt, 1.0)
    m = np.zeros((cfg.HALF, cfg.B), np.float32)
    lo = hf * cfg.HALF
    hi = min(lo + cfg.HALF, cfg.N)
    if hi > lo:
        rows = np.arange(lo, hi)
        m[rows - lo, batch[rows]] = cinv[batch[rows]].astype(np.float32)
    return m.astype(dt)


def _preprocess(inputs, cfg, mm_dt, gs_dt):
    """Build shared program meta + per-core input maps (numpy host work)."""
    f32, bf16 = np.float32, gs_dt
    meta = {}
    shared = {}

    # pretransposed, padded, cast x
    for br in (1, 2):
        x = np.asarray(inputs[f'pro{br}_x'], np.float32)
        xt = np.zeros((cfg.F, cfg.NPAD), mm_dt)
        xt[:, :cfg.N] = x.T.astype(mm_dt)
        shared[f'xT{br}'] = xt
        sched, c_pad, srcs, smat = _edge_plan(cfg, inputs[f'pro{br}_edge_index'])
        meta[f'sched{br}'] = sched
        meta[f'cpad{br}'] = c_pad
        shared[f'_srcs{br}'] = srcs
        shared[f'_smat{br}'] = smat

    # head weights (shared, bf16 for matmuls / f32 biases as [*, 1])
    def colv(v, n):
        return np.asarray(v, np.float32).reshape(n, 1)
    shared['W_pf1'] = np.asarray(inputs['W_pf1'], np.float32)
    shared['W_pf2'] = np.asarray(inputs['W_pf2'], np.float32)
    shared['W_fc1'] = np.asarray(inputs['W_fc1'], np.float32)
    shared['W_fc2'] = np.asarray(inputs['W_fc2'], np.float32)
    wo = np.zeros((256, 1), np.float32)
    wo[0:64] = np.asarray(inputs['W_out'], np.float32)[0:64]
    wo[128:256] = np.asarray(inputs['W_out'], np.float32)[64:192]
    shared['W_out'] = wo
    shared['b_pf1'] = colv(inputs['b_pf1'], 128)
    shared['b_pf2'] = colv(inputs['b_pf2'], 128)
    shared['b_fc1'] = colv(inputs['b_fc1'], 256)
    shared['b_fc2'] = colv(inputs['b_fc2'], 64)
    shared['b_out'] = colv(inputs['b_out'], 1)
    for m in (1, 2):
        shared[f'W_m{m}'] = (np.asarray(inputs[f'W_m{m}'], np.float32)
                             / (2.0 * cfg.LW)).reshape(8, 10, 64)
        shared[f'b_m{m}'] = colv(inputs[f'b_m{m}'], 64)
        for sf, pre in (('s', 'cs'), ('f', 'cf')):
            w = float(np.asarray(inputs[f'{pre}{m}_w'])[0])
            b = float(np.asarray(inputs[f'{pre}{m}_b'])[0])
            shared[f'scale_{sf}{m}'] = np.full((32, 1), w / cfg.C, np.float32)
            shared[f'bias_{sf}{m}'] = np.full((32, 1), b, np.float32)

    in_maps = []
    for core in range(N_CORES):
        fs, hf = core % N_FSLICE, core // N_FSLICE
        f_lo = fs * cfg.FSL
        m = {k: v for k, v in shared.items() if not k.startswith('_')}
        for br in (1, 2):
            W = np.asarray(inputs[f'W_g{br}'], np.float32)[:, f_lo:f_lo + cfg.FSL]
            m[f'Wg{br}'] = np.ascontiguousarray(
                W.reshape(cfg.KC, P, cfg.FSL)).astype(mm_dt)
            bia = np.asarray(inputs[f'b_g{br}'], np.float32)[f_lo:f_lo + cfg.FSL]
            m[f'bg{br}'] = np.tile(bia[None, :], (P, 1)).astype(np.float32)
            m[f'idx{br}'] = _wrap_idxs(shared[f'_srcs{br}'][hf])
            m[f'smat{br}'] = _group_s(shared[f'_smat{br}'][hf], gs_dt)
            m[f'mpool{br}'] = _mpool(cfg, inputs[f'pro{br}_batch'], hf, gs_dt)
        # pooled placement mask [B, N_FSLICE*FSL]
        pm = np.zeros((cfg.B, cfg.F), np.float32)
        pm[:, f_lo:f_lo + cfg.FSL] = 1.0
        m['fmask'] = pm
        # masif slices + placement mask
        gsel = slice(core * cfg.GPB, (core + 1) * cfg.GPB)
        for mi, names in ((1, ('mas1_straight', 'mas1_flipped')),
                          (2, ('mas2_straight', 'mas2_flipped'))):
            m[f'mas{mi}s'] = np.ascontiguousarray(
                np.asarray(inputs[names[0]], np.float32)[gsel])
            m[f'mas{mi}f'] = np.ascontiguousarray(
                np.asarray(inputs[names[1]], np.float32)[gsel])
        mk = np.zeros((P, cfg.B), np.float32)
        mk[:, core * cfg.GPB:(core + 1) * cfg.GPB] = 1.0
        m['gmask'] = mk
        in_maps.append(m)
    return meta, in_maps


# ---------------------------------------------------------------- program
def _build(cfg, meta, mm_dt_np, gs_dt_np):
    import concourse.bass as bass
    import concourse.bacc as bacc
    import concourse.mybir as mybir
    import concourse.tile as tile
    from concourse.masks import make_identity

    dt = mybir.dt
    mm_dt = dt.from_np(np.dtype(mm_dt_np))
    gs_dt = dt.from_np(np.dtype(gs_dt_np))
    f32 = dt.float32
    AF = mybir.ActivationFunctionType
    OP = mybir.AluOpType

    nc = bacc.Bacc("TRN2", target_bir_lowering=False, debug=False,
                   enable_asserts=False, num_devices=N_CORES)

    # ---- dram tensors (inputs)
    def din(name, shape, d):
        return nc.dram_tensor(name, list(shape), d, kind="ExternalInput")

    xT = {br: din(f'xT{br}', (cfg.F, cfg.NPAD), mm_dt) for br in (1, 2)}
    Wg = {br: din(f'Wg{br}', (cfg.KC, P, cfg.FSL), mm_dt) for br in (1, 2)}
    bg = {br: din(f'bg{br}', (P, cfg.FSL), f32) for br in (1, 2)}
    idx = {br: din(f'idx{br}', (P, meta[f'cpad{br}'] * 8), dt.int16)
           for br in (1, 2)}
    smat = {br: din(f'smat{br}', (meta[f'cpad{br}'] // GRP, P, GRP * BLK), gs_dt)
            for br in (1, 2)}
    mpool = {br: din(f'mpool{br}', (cfg.HALF, cfg.B), gs_dt) for br in (1, 2)}
    fmask = din('fmask', (cfg.B, cfg.F), f32)
    gmask = din('gmask', (P, cfg.B), f32)
    mas = {(mi, sf): din(f'mas{mi}{sf}', (cfg.GPB, cfg.C, cfg.L), f32)
           for mi in (1, 2) for sf in 'sf'}
    w_pf = {1: din('W_pf1', (cfg.F, P), f32), 2: din('W_pf2', (cfg.F, P), f32)}
    b_pf = {1: din('b_pf1', (P, 1), f32), 2: din('b_pf2', (P, 1), f32)}
    w_fc1 = din('W_fc1', (256, 256), f32)
    w_fc2 = din('W_fc2', (256, 64), f32)
    b_fc1 = din('b_fc1', (256, 1), f32)
    b_fc2 = din('b_fc2', (64, 1), f32)
    w_out = din('W_out', (256, 1), f32)
    b_out = din('b_out', (1, 1), f32)
    w_m = {mi: din(f'W_m{mi}', (8, 10, 64), f32) for mi in (1, 2)}
    b_m = {mi: din(f'b_m{mi}', (64, 1), f32) for mi in (1, 2)}
    msc = {(mi, sf, kind): din(f'{kind}_{sf}{mi}', (32, 1), f32)
           for mi in (1, 2) for sf in 'sf' for kind in ('scale', 'bias')}

    out_t = nc.dram_tensor('out', [1, cfg.B], f32, kind="ExternalOutput")

    ASM = cfg.B * 2 * cfg.F + P * cfg.B   # allreduce payload (f32 elements)

    with tile.TileContext(nc) as tc:
        with tc.tile_pool(name="const", bufs=1) as cst, \
             tc.tile_pool(name="xt", bufs=2) as xtp, \
             tc.tile_pool(name="xwps", bufs=2, space="PSUM") as xwps, \
             tc.tile_pool(name="xwsb", bufs=3) as xwsb, \
             tc.tile_pool(name="gat", bufs=3) as gatp, \
             tc.tile_pool(name="sld", bufs=3) as sldp, \
             tc.tile_pool(name="idxp", bufs=3) as idxp, \
             tc.tile_pool(name="blkps", bufs=2, space="PSUM") as blkps, \
             tc.tile_pool(name="h블", bufs=4) as hp, \
             tc.tile_pool(name="poolps", bufs=2, space="PSUM") as poolps, \
             tc.tile_pool(name="small", bufs=2) as smp, \
             tc.tile_pool(name="smallps", bufs=1, space="PSUM") as smps, \
             tc.tile_pool(name="dram", bufs=2, space="DRAM") as drp:

            # ---------------- constants into SBUF
            def load(pool, src_ap, shape, d, name=None):
                t = pool.tile(list(shape), d, tag=name)
                nc.sync.dma_start(out=t[:], in_=src_ap)
                return t

            wg_sb = {br: load(cst, Wg[br].ap().transpose([1, 0, 2]),
                              (P, cfg.KC, cfg.FSL), mm_dt, f'wg{br}')
                     for br in (1, 2)}
            bg_sb = {br: load(cst, bg[br][:, :], (P, cfg.FSL), f32, f'bg{br}')
                     for br in (1, 2)}
            mp_sb = {br: load(cst,
                              mpool[br].ap().rearrange(
                                  "(j d) g -> j d g", d=BLK).transpose([1, 0, 2]),
                              (BLK, cfg.NBLK, cfg.B), gs_dt, f'mp{br}')
                     for br in (1, 2)}
            fmask_sb = load(cst, fmask[:, :], (cfg.B, cfg.F), f32, 'fmask')
            gmask_sb = load(cst, gmask[:, :], (P, cfg.B), f32, 'gmask')
            id32 = cst.tile([32, 32], f32, tag='id32')
            make_identity(nc, id32[:])

            wpf_sb = {br: load(cst, w_pf[br].ap().rearrange(
                "(k p) m -> k p m", p=P).transpose([1, 0, 2]),
                               (P, cfg.KC, P), f32, f'wpf{br}') for br in (1, 2)}
            bpf_sb = {br: load(cst, b_pf[br][:, :], (P, 1), f32, f'bpf{br}')
                      for br in (1, 2)}
            wfc1_sb = load(cst, w_fc1.ap().rearrange(
                "(a p) m -> a p m", p=P).transpose([1, 0, 2]),
                           (P, 2, 256), f32, 'wfc1')
            wfc2_sb = load(cst, w_fc2.ap().rearrange(
                "(a p) m -> a p m", p=P).transpose([1, 0, 2]),
                           (P, 2, 64), f32, 'wfc2')
            bfc1_sb = load(cst, b_fc1.ap().rearrange(
                "(a p) m -> a p m", p=P).transpose([1, 0, 2]),
                           (P, 2, 1), f32, 'bfc1')
            bfc2_sb = load(cst, b_fc2[:, :], (64, 1), f32, 'bfc2')
            wout_sb = load(cst, w_out.ap().rearrange(
                "(a p) m -> a p m", p=P).transpose([1, 0, 2]),
                           (P, 2, 1), f32, 'wout')
            bout_sb = load(cst, b_out[:, :], (1, 1), f32, 'bout')
            wm_sb = {mi: load(cst, w_m[mi].ap().transpose([1, 0, 2]),
                              (10, 8, 64), f32, f'wm{mi}') for mi in (1, 2)}
            bm_sb = {mi: load(cst, b_m[mi][:, :], (64, 1), f32, f'bm{mi}')
                     for mi in (1, 2)}
            msc_sb = {k: load(cst, v[:, :], (32, 1), f32, f'msc{k}')
                      for k, v in msc.items()}

            # ---------------- masif (both branches) -> two [64, B] f32 tiles
            masif_asm1 = cst.tile([64, cfg.B], f32, tag='masasm1')
            masif_asm2 = cst.tile([64, cfg.B], f32, tag='masasm2')
            masif_asm = {1: masif_asm1, 2: masif_asm2}
            for mi in (1, 2):
                frag = None
                for sf in 'sf':
                    src = mas[(mi, sf)]
                    t = smp.tile([32, cfg.C, cfg.LBS], f32, tag='masload')
                    for lb in range(cfg.LB):
                        nc.sync.dma_start(
                            out=t[lb * cfg.GPB:(lb + 1) * cfg.GPB],
                            in_=src.ap()[:, :, lb * cfg.LBS:(lb + 1) * cfg.LBS])
                    red = smp.tile([32, cfg.LBS], f32, tag='masred')
                    nc.vector.tensor_reduce(
                        out=red[:], in_=t[:].transpose([0, 2, 1]),
                        axis=mybir.AxisListType.X, op=OP.add)
                    act = smp.tile([32, cfg.LBS], f32, tag='masact')
                    nc.scalar.activation(
                        act[:], red[:], AF.Relu,
                        bias=msc_sb[(mi, sf, 'bias')][:, 0:1],
                        scale=msc_sb[(mi, sf, 'scale')][:, 0:1])
                    ws = smp.tile([32, cfg.WPB], f32, tag='masws')
                    nc.vector.tensor_reduce(
                        out=ws[:],
                        in_=act[:].rearrange("p (w l) -> p w l", l=cfg.LW),
                        axis=mybir.AxisListType.X, op=OP.add)
                    if frag is None:
                        frag = ws
                    else:
                        frag2 = smp.tile([32, cfg.WPB], f32, tag='masfrag')
                        nc.vector.tensor_add(out=frag2[:], in0=frag[:], in1=ws[:])
                        frag = frag2
                # transpose [32, 10] -> [10, 32]
                ps_t = smps.tile([cfg.WPB, 32], f32, space="PSUM", tag='sps')
                nc.tensor.transpose(out=ps_t[:], in_=frag[:], identity=id32[:])
                fragT = smp.tile([cfg.WPB, 32], f32, tag='masfragT')
                nc.scalar.activation(fragT[:], ps_t[:], AF.Identity)
                fragTc = fragT[:].rearrange("k (lb g) -> k lb g", g=cfg.GPB)
                m_ps = smps.tile([64, cfg.GPB], f32, space="PSUM", tag='spsacc')
                for lb in range(cfg.LB):
                    nc.tensor.matmul(
                        m_ps[:], lhsT=wm_sb[mi][:, lb, :], rhs=fragTc[:, lb, :],
                        start=(lb == 0), stop=(lb == cfg.LB - 1))
                m_fm = smp.tile([64, cfg.GPB], f32, tag='masfm')
                nc.scalar.activation(m_fm[:], m_ps[:], AF.Identity,
                                     bias=bm_sb[mi][:, 0:1])
                # broadcast to [64, B] * gmask -> masif_asm[mi]
                nc.vector.tensor_tensor(
                    out=masif_asm[mi][:].rearrange(
                        "p (s g) -> p s g", g=cfg.GPB),
                    in0=m_fm[:, None, :].to_broadcast(
                        [64, N_CORES, cfg.GPB]),
                    in1=gmask_sb[0:64, :].rearrange(
                        "p (s g) -> p s g", g=cfg.GPB),
                    op=OP.mult)

            # ---------------- GCN branches
            pooled_full = cst.tile([cfg.B, 2 * cfg.F], f32, tag='poolfull')
            NT = cfg.NPAD // 512
            for br in (1, 2):
                cpad = meta[f'cpad{br}']
                sched = meta[f'sched{br}']
                xw_dram = drp.tile([cfg.NPAD, cfg.FSL], gs_dt, tag='xwdram')
                # xw = x @ W slice
                for nt in range(NT):
                    xt_t = xtp.tile([P, cfg.KC, 512], mm_dt, tag='xt')
                    for k in range(cfg.KC):
                        nc.sync.dma_start(
                            out=xt_t[:, k, :],
                            in_=xT[br][k * P:(k + 1) * P,
                                       nt * 512:(nt + 1) * 512])
                    for sub in range(4):
                        ps = xwps.tile([P, cfg.FSL], f32, space="PSUM",
                                       tag='xwps')
                        for k in range(cfg.KC):
                            nc.tensor.matmul(
                                ps[:],
                                lhsT=xt_t[:, k, sub * P:(sub + 1) * P],
                                rhs=wg_sb[br][:, k, :],
                                start=(k == 0), stop=(k == cfg.KC - 1))
                        xw_t = xwsb.tile([P, cfg.FSL], gs_dt, tag='xwsb')
                        nc.scalar.activation(xw_t[:], ps[:], AF.Identity)
                        nc.sync.dma_start(
                            out=xw_dram[(nt * 4 + sub) * P:
                                        (nt * 4 + sub + 1) * P, :],
                            in_=xw_t[:])

                # scatter + pool
                pool_ps = poolps.tile([cfg.B, cfg.FSL], f32, space="PSUM",
                                      tag='poolps')
                n_grp = cpad // GRP
                ci = 0
                blk_ps = None
                for g in range(n_grp):
                    idx_t = idxp.tile([P, GRP * 8], dt.int16, tag='idx')
                    nc.sync.dma_start(
                        out=idx_t[:],
                        in_=idx[br][:, g * GRP * 8:(g + 1) * GRP * 8])
                    gat_t = gatp.tile([P, GRP, cfg.FSL], gs_dt, tag='gat')
                    nc.gpsimd.dma_gather(
                        out_ap=gat_t[:], in_ap=xw_dram[:, :], idxs_ap=idx_t[:],
                        num_idxs=GRP * P, num_idxs_reg=GRP * P,
                        elem_size=cfg.FSL)
                    s_t = sldp.tile([P, GRP * BLK], gs_dt, tag='sld')
                    nc.sync.dma_start(out=s_t[:], in_=smat[br][g, :, :])
                    for i in range(GRP):
                        j, st, sp = sched[ci]
                        if st:
                            blk_ps = blkps.tile([BLK, cfg.FSL], f32,
                                                space="PSUM", tag='blkps')
                        nc.tensor.matmul(
                            blk_ps[:],
                            lhsT=s_t[:, i * BLK:(i + 1) * BLK],
                            rhs=gat_t[:, i, :],
                            start=st, stop=sp)
                        if sp:
                            h_t = hp.tile([BLK, cfg.FSL], gs_dt, tag='h')
                            nc.vector.tensor_add(out=h_t[:], in0=blk_ps[:],
                                                 in1=bg_sb[br][0:BLK, :])
                            nc.scalar.activation(h_t[:], h_t[:], AF.Lrelu,
                                                 alpha=0.01)
                            nc.tensor.matmul(
                                pool_ps[:], lhsT=mp_sb[br][:, j, :],
                                rhs=h_t[:],
                                start=(j == 0), stop=(j == cfg.NBLK - 1))
                        ci += 1
                # pooled [B, FSL] -> mask-place into pooled_full
                pooled_sb = smp.tile([cfg.B, cfg.FSL], f32, tag='pooled')
                nc.scalar.activation(pooled_sb[:], pool_ps[:], AF.Identity)
                nc.vector.tensor_tensor(
                    out=pooled_full[:, (br - 1) * cfg.F:br * cfg.F].rearrange(
                        "p (s m) -> p s m", m=cfg.FSL),
                    in0=pooled_sb[:, None, :].to_broadcast(
                        [cfg.B, N_FSLICE, cfg.FSL]),
                    in1=fmask_sb[:, :].rearrange("p (s m) -> p s m", m=cfg.FSL),
                    op=OP.mult)

            # ---------------- allreduce
            bounce_in = drp.tile([ASM], f32, tag='ccin')
            bounce_out = drp.tile([ASM], f32, tag='ccout')
            pf_n = cfg.B * 2 * cfg.F
            nc.sync.dma_start(
                out=bounce_in[0:pf_n].rearrange("(p f) -> p f", f=2 * cfg.F),
                in_=pooled_full[:])
            half_m = 64 * cfg.B
            for mi in (1, 2):
                lo = pf_n + (mi - 1) * half_m
                nc.sync.dma_start(
                    out=bounce_in[lo:lo + half_m].rearrange(
                        "(p f) -> p f", f=cfg.B),
                    in_=masif_asm[mi][:])
            nc.gpsimd.collective_compute(
                "AllReduce", OP.add,
                replica_groups=[list(range(N_CORES))],
                ins=[bounce_in[:].opt()], outs=[bounce_out[:].opt()])

            pooled_gm = smp.tile([cfg.B, 2 * cfg.F], f32, tag='poolgm')
            nc.sync.dma_start(
                out=pooled_gm[:],
                in_=bounce_out[0:pf_n].rearrange("(p f) -> p f", f=2 * cfg.F))
            masif_rb = smp.tile([P, cfg.B], f32, tag='masrb')
            nc.sync.dma_start(
                out=masif_rb[:],
                in_=bounce_out[pf_n:ASM].rearrange("(p f) -> p f", f=cfg.B))

            # ---------------- head (replicated on all cores)
            x12 = {}
            for br in (1, 2):
                pfm = smp.tile([P, cfg.KC, cfg.B], f32, tag=f'pfm{br}')
                for k in range(cfg.KC):
                    tps = smps.tile([P, cfg.B], f32, space="PSUM", tag='sps')
                    nc.tensor.transpose(
                        out=tps[:],
                        in_=pooled_gm[:, (br - 1) * cfg.F + k * P:
                                      (br - 1) * cfg.F + (k + 1) * P],
                        identity=id32[:])
                    nc.scalar.activation(pfm[:, k, :], tps[:], AF.Identity)
                xps = smps.tile([P, cfg.B], f32, space="PSUM", tag='spsacc')
                for k in range(cfg.KC):
                    nc.tensor.matmul(xps[:], lhsT=wpf_sb[br][:, k, :],
                                     rhs=pfm[:, k, :],
                                     start=(k == 0), stop=(k == cfg.KC - 1))
                xs = smp.tile([P, cfg.B], f32, tag=f'x{br}')
                nc.scalar.activation(xs[:], xps[:], AF.Lrelu,
                                     bias=bpf_sb[br][:, 0:1], alpha=0.01)
                x12[br] = xs

            xc1 = {}
            for mh in range(2):
                cps = smps.tile([P, cfg.B], f32, space="PSUM", tag='spsacc')
                for k2 in range(2):
                    nc.tensor.matmul(
                        cps[:], lhsT=wfc1_sb[:, k2, mh * P:(mh + 1) * P],
                        rhs=x12[k2 + 1][:], start=(k2 == 0), stop=(k2 == 1))
                xcs = smp.tile([P, cfg.B], f32, tag=f'xc{mh}')
                nc.scalar.activation(xcs[:], cps[:], AF.Lrelu,
                                     bias=bfc1_sb[:, mh, 0:1], alpha=0.01)
                xc1[mh] = xcs
            c2ps = smps.tile([64, cfg.B], f32, space="PSUM", tag='spsacc')
            for k2 in range(2):
                nc.tensor.matmul(c2ps[:], lhsT=wfc2_sb[:, k2, :],
                                 rhs=xc1[k2][:], start=(k2 == 0), stop=(k2 == 1))
            xc = smp.tile([64, cfg.B], f32, tag='xcf')
            nc.scalar.activation(xc[:], c2ps[:], AF.Lrelu,
                                 bias=bfc2_sb[:, 0:1], alpha=0.01)

            ops = smps.tile([1, cfg.B], f32, space="PSUM", tag='spsacc')
            nc.tensor.matmul(ops[:], lhsT=wout_sb[0:64, 0, :], rhs=xc[:],
                             start=True, stop=False)
            nc.tensor.matmul(ops[:], lhsT=wout_sb[:, 1, :], rhs=masif_rb[:],
                             start=False, stop=True)
            res = smp.tile([1, cfg.B], f32, tag='res')
            nc.scalar.activation(res[:], ops[:], AF.Sigmoid,
                                 bias=bout_sb[:, 0:1])
            nc.sync.dma_start(out=out_t[:, :], in_=res[:])

    nc.compile()
    return nc


# ---------------------------------------------------------------- entry
_CACHE = {}


def _run(inputs, cfg, mm_dt=None, gs_dt=None, trace=False, tmpdir=None):
    import ml_dtypes
    from concourse import bass_utils
    mm_dt = mm_dt or ml_dtypes.bfloat16
    gs_dt = gs_dt or ml_dtypes.bfloat16
    meta, in_maps = _preprocess(inputs, cfg, mm_dt, gs_dt)
    key = (cfg.N, cfg.F, meta['cpad1'], meta['cpad2'],
           tuple(x[0] for x in meta['sched1']),
           tuple(x[0] for x in meta['sched2']),
           np.dtype(mm_dt).name, np.dtype(gs_dt).name)
    if key not in _CACHE:
        _CACHE.clear()
        _CACHE[key] = _build(cfg, meta, mm_dt, gs_dt)
    nc = _CACHE[key]
    res = bass_utils.run_bass_kernel_spmd(
        nc, in_maps, core_ids=list(range(N_CORES)), trace=trace, tmpdir=tmpdir)
    out = np.asarray(res.results[0]['out'], np.float32).reshape(cfg.B, 1)
    return out, res


def kernel(**inputs) -> np.ndarray:
    cfg = _Cfg()
    out, _ = _run(inputs, cfg)
    return out



# revision 7
# speedup vs baseline: 1.6117x; 1.6117x over previous
"""Trainium2 Bass kernel for nn_GCNN_87668872446200.

Branch-split design over 8 NeuronCores: cores 0-3 run protein branch 1,
cores 4-7 run branch 2.  Within a branch group each core owns a quarter of
the destination nodes and the full F=1024 feature dim.

Per core (fp8 e4m3 data paths, DoubleRow fp8 matmuls):
  - xw' = 8*(x*dinv_row) @ (W*64) / 8   computed on PE in two source-halves,
    written to HBM as two tensors (xwA rows <5120 + bias row, xwB rest)
  - symmetric norm is separated: h = Dinv A Dinv xw + b realized as
    S-matmul with S[e,d] = dinv[d] (bias via a virtual edge to a bias row)
  - dma_gather pulls 1KB fp8 rows per edge; source-half split lets the
    Q7 descriptor emission of half A overlap the xw compute of half B
  - phase A partial sums staged in SBUF (fp8), injected into phase B PSUM
    via an identity matmul; one ACT pass does lrelu(psum/8)
  - per-graph mean-pool as PE matmul (mpool*256 fp8), W_pf applied locally
  - masif branch (8 graphs/core, this core's branch only)
  - one small AllReduce ([3,128,32] f32 = 48KB) + replicated dense head

All 8 cores run ONE identical program; per-core variation is in input data.
"""
import numpy as np

# ---------------------------------------------------------------- constants
N_CORES = 8
P = 128
BLK = 128           # dest nodes per block
NQ = 4              # dest quarters per branch group
GRPU = 8            # 128-idx units per gather call (1024 idxs)

N_NODES, N_EDGES, F_DIM, B_GRAPHS, L_MAS, C_MAS = 10000, 80000, 1024, 32, 800, 16

USE_DR = True       # DoubleRow fp8 matmuls


class _Cfg:
    def __init__(self, n=N_NODES, e=N_EDGES, f=F_DIM, b=B_GRAPHS,
                 l=L_MAS, c=C_MAS):
        self.N, self.E, self.F, self.B, self.L, self.C = n, e, f, b, l, c
        self.NPAD = ((n + 511) // 512) * 512          # 10240
        self.QH = self.NPAD // NQ                     # 2560 dests per core
        self.NBLK = self.QH // BLK                    # 20 blocks
        self.SH = self.NPAD // 2                      # 5120 source-half split
        self.KC2 = f // 256                           # 4 k-pairs
        self.GPB = b // 4                             # 8 graphs per core
        self.LW = l // 80                             # 10
        self.LB = 8                                   # l-blocks
        self.LBS = l // self.LB                       # 100
        self.WPB = self.LBS // self.LW                # 10
        # xwA holds source rows 0..SH-1 plus bias row (SH) and zero row (SH+1)
        self.XWA_ROWS = self.SH + P                   # 5248
        self.XWB_ROWS = self.NPAD - self.SH           # 5120 (tail rows zero)


def _q8(x):
    import ml_dtypes
    return np.clip(np.asarray(x, np.float32), -240.0, 240.0).astype(
        ml_dtypes.float8_e4m3)


# ---------------------------------------------------------------- host prep
def _edge_plan_core(cfg, edge_index, q):
    """Edges targeting quarter q, split per (block, source-half), sorted.
    Returns dict (j, hf) -> (rows, dests, counts)."""
    row = np.asarray(edge_index[0]).astype(np.int64)
    col = np.asarray(edge_index[1]).astype(np.int64)
    loops = np.arange(cfg.N, dtype=np.int64)
    rows = np.concatenate([row, loops])
    cols = np.concatenate([col, loops])
    lo, hi = q * cfg.QH, (q + 1) * cfg.QH
    sel = (cols >= lo) & (cols < hi)
    r, c = rows[sel], cols[sel] - lo
    out = {}
    for j in range(cfg.NBLK):
        bsel = (c >= j * BLK) & (c < (j + 1) * BLK)
        rj, cj = r[bsel], c[bsel] - j * BLK
        for hf in range(2):
            hsel = (rj < cfg.SH) if hf == 0 else (rj >= cfg.SH)
            out[(j, hf)] = (rj[hsel], cj[hsel])
    return out


def _shared_schedule(cfg, plans):
    """Shared chunk schedule (max over the 8 per-core plans).

    Returns chunks: list of dicts with keys
      hf, j, units (1 or 2), u0 (unit offset in group), grp (group index),
      first (starts block), last (ends block's half... block completion is
      tracked at (j,hf==1,last) for B and (j,hf==0,last) for A)
    and n_groups_a / n_groups_b.
    """
    need = {}
    for j in range(cfg.NBLK):
        for hf in range(2):
            e_max = max(len(p[(j, hf)][0]) for p in plans)
            if hf == 0:
                e_max += 1                      # bias slot
            n256 = e_max // 256
            rem = e_max - 256 * n256
            n128 = 0
            if rem > 128:
                n256 += 1
            elif rem > 0:
                n128 = 1
            if n256 == 0 and n128 == 0:
                n128 = 1
            need[(j, hf)] = (n256, n128)

    chunks = []
    groups = []                                   # list of (hf, nunits)
    for hf in range(2):
        ucur = GRPU                               # force new group
        for j in range(cfg.NBLK):
            n256, n128 = need[(j, hf)]
            sizes = [2] * n256 + [1] * n128
            for k, sz in enumerate(sizes):
                if ucur + sz > GRPU:
                    groups.append([hf, 0])
                    ucur = 0
                chunks.append(dict(hf=hf, j=j, units=sz, u0=ucur,
                                   grp=len(groups) - 1,
                                   first=(k == 0), last=(k == len(sizes) - 1)))
                ucur += sz
                groups[-1][1] = ucur
    return chunks, groups


def _fill_core_gather2(cfg, chunks, groups, plan, dinv, q):
    """Per-core idx + smat content for the shared schedule (correct
    multi-chunk consumption)."""
    n_groups = len(groups)
    flat_idx = np.zeros((n_groups, GRPU * P), np.int64)
    smat = np.zeros((n_groups, P, GRPU * P), np.float32)
    dinv8 = _q8(dinv).astype(np.float32)
    consumed = {}
    for ch in chunks:
        j, hf, g, u0 = ch['j'], ch['hf'], ch['grp'], ch['u0']
        r, c = plan[(j, hf)]
        off = consumed.get((j, hf), 0)
        cap = ch['units'] * P
        base = u0 * P
        pad_idx = cfg.SH + 1 if hf == 0 else cfg.XWB_ROWS - 1
        flat_idx[g, base:base + cap] = pad_idx
        s = 0
        if hf == 0 and ch['first']:
            flat_idx[g, base] = cfg.SH            # bias row at slot 0
            smat[g, 0, base:base + P] = 1.0
            s = 1
        take = min(len(r) - off, cap - s)
        if take > 0:
            rr = r[off:off + take]
            cc = c[off:off + take]
            if hf == 1:
                rr = rr - cfg.SH
            slots = np.arange(s, s + take) + base
            up = slots // P
            pp = slots % P
            flat_idx[g, slots] = rr
            # dest scale dinv[global dest] ; global dest = q*QH + j*BLK + cc
            gd = q * cfg.QH + j * BLK + cc
            vals = dinv8[np.minimum(gd, cfg.N - 1)] * (gd < cfg.N)
            smat[g, pp, up * P + cc] = vals
        consumed[(j, hf)] = off + take
    for (j, hf), off in consumed.items():
        assert off == len(plan[(j, hf)][0]), (j, hf, off, len(plan[(j, hf)][0]))
    return flat_idx, smat


def _wrap_idx_groups(flat_idx):
    """[G, 1024] -> [128, G*64] int16 (16-part wrap, 8x replicated)."""
    g, n = flat_idx.shape
    w = flat_idx.reshape(g, n // 16, 16).transpose(2, 0, 1).reshape(16, -1)
    return np.tile(w, (8, 1)).astype(np.int16)


def _preprocess(inputs, cfg):
    import ml_dtypes
    bf16 = ml_dtypes.bfloat16
    f32 = np.float32

    # --- per-branch shared data
    bdata = {}
    for br in (1, 2):
        x = np.asarray(inputs[f'pro{br}_x'], f32)
        ei = np.asarray(inputs[f'pro{br}_edge_index'])
        batch = np.asarray(inputs[f'pro{br}_batch']).astype(np.int64)
        row = ei[0].astype(np.int64)
        col = ei[1].astype(np.int64)
        deg = np.bincount(np.concatenate([col, np.arange(cfg.N)]),
                          minlength=cfg.N).astype(np.float64)
        dinv = (1.0 / np.sqrt(deg)).astype(f32)
        # xT_dr [4, 128, 2, NPAD] fp8 of (x*dinv_row)^T
        xp = x * dinv[:, None]
        xpT = np.zeros((cfg.F, cfg.NPAD), f32)
        xpT[:, :cfg.N] = xp.T
        xt_dr = np.ascontiguousarray(
            xpT.reshape(cfg.KC2, 2, P, cfg.NPAD).transpose(0, 2, 1, 3))
        W = np.asarray(inputs[f'W_g{br}'], f32) * 64.0
        wg_dr = np.ascontiguousarray(
            W.reshape(cfg.KC2, 2, P, cfg.F).transpose(0, 2, 1, 3))
        b8 = np.asarray(inputs[f'b_g{br}'], f32) * 8.0
        cnt = np.bincount(batch, minlength=cfg.B).astype(f32)
        plans = [_edge_plan_core(cfg, ei, q) for q in range(NQ)]
        bdata[br] = dict(xt=_q8(xt_dr), wg=_q8(wg_dr), b8=_q8(b8[None, :]),
                         dinv=dinv, batch=batch, cnt=cnt, plans=plans)

    # --- shared chunk schedule (max over all 8 core plans)
    all_plans = bdata[1]['plans'] + bdata[2]['plans']
    chunks, groups = _shared_schedule(cfg, all_plans)
    n_groups = len(groups)
    n_ga = sum(1 for g in groups if g[0] == 0)

    meta = dict(chunks=chunks, groups=groups, n_groups=n_groups, n_ga=n_ga)

    # --- head weights (shared across cores)
    shared = {}
    shared['wfc1'] = np.ascontiguousarray(
        np.asarray(inputs['W_fc1'], f32).reshape(2, P, 256).transpose(1, 0, 2))
    shared['wfc2'] = np.ascontiguousarray(
        np.asarray(inputs['W_fc2'], f32).reshape(2, P, 64).transpose(1, 0, 2))
    shared['bfc1'] = np.ascontiguousarray(
        np.asarray(inputs['b_fc1'], f32).reshape(2, P, 1).transpose(1, 0, 2))
    shared['bfc2'] = np.asarray(inputs['b_fc2'], f32).reshape(64, 1)
    wout = np.asarray(inputs['W_out'], f32)
    shared['wout_x'] = np.ascontiguousarray(wout[0:64])            # [64,1]
    shared['wout_m'] = np.ascontiguousarray(wout[64:192])          # [128,1]
    shared['bout'] = np.asarray(inputs['b_out'], f32).reshape(1, 1)
    shared['bpf1'] = np.asarray(inputs['b_pf1'], f32).reshape(P, 1)
    shared['bpf2'] = np.asarray(inputs['b_pf2'], f32).reshape(P, 1)
    shared['id32'] = np.eye(32, dtype=f32)
    shared['id128_8'] = _q8(np.eye(P, dtype=f32))
    shared['id64'] = np.eye(64, dtype=f32)

    in_maps = []
    for core in range(N_CORES):
        br = 1 + core // NQ
        q = core % NQ
        bd = bdata[br]
        m = dict(shared)
        m['xt'] = bd['xt']
        m['wg'] = bd['wg']
        m['b8row'] = bd['b8']
        # gather plan
        flat_idx, smat = _fill_core_gather2(
            cfg, chunks, groups, bd['plans'][q], bd['dinv'], q)
        m['idx'] = _wrap_idx_groups(flat_idx)
        m['smat'] = np.ascontiguousarray(
            smat.transpose(1, 0, 2).reshape(P, n_groups * GRPU * P)).astype(
            ml_dtypes.float8_e4m3)
        # mpool [128, NBLK, B] fp8 (x256)
        mp = np.zeros((P, cfg.NBLK, cfg.B), f32)
        for j in range(cfg.NBLK):
            nodes = q * cfg.QH + j * BLK + np.arange(BLK)
            ok = nodes < cfg.N
            gidx = bd['batch'][np.minimum(nodes, cfg.N - 1)]
            val = 256.0 / np.maximum(bd['cnt'][gidx], 1.0) * ok
            mp[np.arange(BLK), j, gidx] = val
        m['mpool'] = _q8(mp.reshape(P, cfg.NBLK * cfg.B))
        # W_pf for this branch  [8, 128, 128] f32  (k-chunk major)
        wpf = np.asarray(inputs[f'W_pf{br}'], f32)
        m['wpf'] = np.ascontiguousarray(wpf.reshape(8, P, P))
        # branch masks for cc packing
        m['mask1'] = np.full((P, 1), 1.0 if br == 1 else 0.0, f32)
        m['mask2'] = np.full((P, 1), 1.0 if br == 2 else 0.0, f32)
        # masif (this branch only, 8 graphs)
        gs = (core % NQ) * cfg.GPB
        m['mas_s'] = np.ascontiguousarray(
            np.asarray(inputs[f'mas{br}_straight'], f32)[gs:gs + cfg.GPB])
        m['mas_f'] = np.ascontiguousarray(
            np.asarray(inputs[f'mas{br}_flipped'], f32)[gs:gs + cfg.GPB])
        # wm128 [10, 8, 128]: this branch's W_m/(2*LW) at rows 64*(br-1)..
        wm = np.zeros((cfg.WPB, cfg.LB, P), f32)
        wsrc = (np.asarray(inputs[f'W_m{br}'], f32) / (2.0 * cfg.LW)).reshape(
            cfg.LB, cfg.WPB, 64)
        wm[:, :, 64 * (br - 1):64 * br] = wsrc.transpose(1, 0, 2)
        m['wm'] = np.ascontiguousarray(wm)
        bm = np.zeros((P, 1), f32)
        bm[64 * (br - 1):64 * br, 0] = np.asarray(inputs[f'b_m{br}'], f32)
        m['bm'] = bm
        gm = np.zeros((P, cfg.B), f32)
        gm[64 * (br - 1):64 * br, gs:gs + cfg.GPB] = 1.0
        m['gmask'] = gm
        for sf, pre in (('s', 'cs'), ('f', 'cf')):
            w = float(np.asarray(inputs[f'{pre}{br}_w'])[0])
            b = float(np.asarray(inputs[f'{pre}{br}_b'])[0])
            m[f'scale_{sf}'] = np.full((64, 1), w / cfg.C, f32)
            m[f'bias_{sf}'] = np.full((64, 1), b, f32)
        in_maps.append(m)
    return meta, in_maps


# ---------------------------------------------------------------- program
def _build(cfg, meta):
    import concourse.bass as bass
    import concourse.bacc as bacc
    import concourse.mybir as mybir
    import concourse.tile as tile

    dt = mybir.dt
    f32 = dt.float32
    fp8 = dt.float8e4
    AF = mybir.ActivationFunctionType
    OP = mybir.AluOpType
    DR = mybir.MatmulPerfMode.DoubleRow if USE_DR else None

    chunks = meta['chunks']
    groups = meta['groups']
    n_groups = meta['n_groups']
    n_ga = meta['n_ga']

    nc = bacc.Bacc("TRN2", target_bir_lowering=False, debug=False,
                   enable_asserts=False, num_devices=N_CORES)

    def din(name, shape, d):
        return nc.dram_tensor(name, list(shape), d, kind="ExternalInput")

    xt_d = din('xt', (cfg.KC2, P, 2, cfg.NPAD), fp8)
    wg_d = din('wg', (cfg.KC2, P, 2, cfg.F), fp8)
    b8_d = din('b8row', (1, cfg.F), fp8)
    idx_d = din('idx', (P, n_groups * 64), dt.int16)
    smat_d = din('smat', (P, n_groups * GRPU * P), fp8)
    mpool_d = din('mpool', (P, cfg.NBLK * cfg.B), fp8)
    wpf_d = din('wpf', (8, P, P), f32)
    mas_d = {sf: din(f'mas_{sf}', (cfg.GPB, cfg.C, cfg.L), f32) for sf in 'sf'}
    wm_d = din('wm', (cfg.WPB, cfg.LB, P), f32)
    bm_d = din('bm', (P, 1), f32)
    gmask_d = din('gmask', (P, cfg.B), f32)
    msc_d = {(sf, kind): din(f'{kind}_{sf}', (64, 1), f32)
             for sf in 'sf' for kind in ('scale', 'bias')}
    wfc1_d = din('wfc1', (P, 2, 256), f32)
    wfc2_d = din('wfc2', (P, 2, 64), f32)
    bfc1_d = din('bfc1', (P, 2, 1), f32)
    bfc2_d = din('bfc2', (64, 1), f32)
    wout_x_d = din('wout_x', (64, 1), f32)
    wout_m_d = din('wout_m', (P, 1), f32)
    bout_d = din('bout', (1, 1), f32)
    bpf1_d = din('bpf1', (P, 1), f32)
    bpf2_d = din('bpf2', (P, 1), f32)
    mask1_d = din('mask1', (P, 1), f32)
    mask2_d = din('mask2', (P, 1), f32)
    id32_d = din('id32', (32, 32), f32)
    id64_d = din('id64', (64, 64), f32)
    id128_d = din('id128_8', (P, P), fp8)

    out_t = nc.dram_tensor('out', [1, cfg.B], f32, kind="ExternalOutput")

    CC = 3 * P * cfg.B          # allreduce payload (f32 elements)

    with tile.TileContext(nc) as tc:
        with tc.tile_pool(name="const", bufs=1) as cst, \
             tc.tile_pool(name="xt", bufs=2) as xtp, \
             tc.tile_pool(name="xwsb", bufs=3) as xwsb, \
             tc.tile_pool(name="gat", bufs=3) as gatp, \
             tc.tile_pool(name="hsb", bufs=2) as hp, \
             tc.tile_pool(name="small", bufs=2) as smp, \
             tc.tile_pool(name="psA", bufs=2, space="PSUM") as psA, \
             tc.tile_pool(name="psBlk", bufs=1, space="PSUM") as psB, \
             tc.tile_pool(name="psPool", bufs=1, space="PSUM") as psP, \
             tc.tile_pool(name="psX", bufs=2, space="PSUM") as psX, \
             tc.tile_pool(name="dram", bufs=1, space="DRAM") as drp:

            def load(pool, src_ap, shape, d, name=None):
                t = pool.tile(list(shape), d, tag=name)
                nc.sync.dma_start(out=t[:], in_=src_ap)
                return t

            # ---------------- constants
            wg_sb = load(cst, wg_d[:, :, :, :].transpose([1, 0, 2, 3]),
                         (P, cfg.KC2, 2, cfg.F), fp8, 'wg')
            idx_sb = load(cst, idx_d[:, :], (P, n_groups * 64), dt.int16, 'idx')
            smat_sb = load(cst, smat_d[:, :], (P, n_groups * GRPU * P), fp8,
                           'smat')
            mpool_sb = load(cst, mpool_d[:, :], (P, cfg.NBLK * cfg.B), fp8,
                            'mpool')
            wpf_sb = load(cst, wpf_d.ap().transpose([1, 0, 2]), (P, 8, P), f32,
                          'wpf')
            wm_sb = load(cst, wm_d[:, :, :], (cfg.WPB, cfg.LB, P), f32, 'wm')
            bm_sb = load(cst, bm_d[:, :], (P, 1), f32, 'bm')
            gmask_sb = load(cst, gmask_d[:, :], (P, cfg.B), f32, 'gmask')
            msc_sb = {k: load(cst, v[:, :], (64, 1), f32, f'msc{k}')
                      for k, v in msc_d.items()}
            wfc1_sb = load(cst, wfc1_d[:, :, :], (P, 2, 256), f32, 'wfc1')
            wfc2_sb = load(cst, wfc2_d[:, :, :], (P, 2, 64), f32, 'wfc2')
            bfc1_sb = load(cst, bfc1_d[:, :, :], (P, 2, 1), f32, 'bfc1')
            bfc2_sb = load(cst, bfc2_d[:, :], (64, 1), f32, 'bfc2')
            wout_x_sb = load(cst, wout_x_d[:, :], (64, 1), f32, 'woutx')
            wout_m_sb = load(cst, wout_m_d[:, :], (P, 1), f32, 'woutm')
            bout_sb = load(cst, bout_d[:, :], (1, 1), f32, 'bout')
            bpf1_sb = load(cst, bpf1_d[:, :], (P, 1), f32, 'bpf1')
            bpf2_sb = load(cst, bpf2_d[:, :], (P, 1), f32, 'bpf2')
            mask1_sb = load(cst, mask1_d[:, :], (P, 1), f32, 'mask1')
            mask2_sb = load(cst, mask2_d[:, :], (P, 1), f32, 'mask2')
            id32 = load(cst, id32_d[:, :], (32, 32), f32, 'id32')
            id64 = load(cst, id64_d[:, :], (64, 64), f32, 'id64')
            id128 = load(cst, id128_d[:, :], (P, P), fp8, 'id128')
            b8_sb = load(cst, b8_d[:, :], (1, cfg.F), fp8, 'b8')

            hA = cst.tile([P, cfg.NBLK * cfg.F], fp8, tag='hA')

            # ---------------- masif (one branch, 8 graphs -> [128, B] via PE)
            frag = None
            for sf in 'sf':
                src = mas_d[sf]
                t = smp.tile([64, cfg.C, cfg.LBS], f32, tag='masload')
                for lb in range(cfg.LB):
                    nc.sync.dma_start(
                        out=t[lb * cfg.GPB:(lb + 1) * cfg.GPB],
                        in_=src.ap()[:, :, lb * cfg.LBS:(lb + 1) * cfg.LBS])
                red = smp.tile([64, cfg.LBS], f32, tag='masred')
                nc.vector.tensor_reduce(
                    out=red[:], in_=t[:].transpose([0, 2, 1]),
                    axis=mybir.AxisListType.X, op=OP.add)
                act = smp.tile([64, cfg.LBS], f32, tag='masact')
                nc.scalar.activation(
                    act[:], red[:], AF.Relu,
                    bias=msc_sb[(sf, 'bias')][:, 0:1],
                    scale=msc_sb[(sf, 'scale')][:, 0:1])
                ws = smp.tile([64, cfg.WPB], f32, tag='masws')
                nc.vector.tensor_reduce(
                    out=ws[:],
                    in_=act[:].rearrange("p (w l) -> p w l", l=cfg.LW),
                    axis=mybir.AxisListType.X, op=OP.add)
                if frag is None:
                    frag = ws
                else:
                    frag2 = smp.tile([64, cfg.WPB], f32, tag='masfrag')
                    nc.vector.tensor_add(out=frag2[:], in0=frag[:], in1=ws[:])
                    frag = frag2
            ps_t = psX.tile([cfg.WPB, 64], f32, space="PSUM", tag='aux')
            nc.tensor.transpose(out=ps_t[:], in_=frag[:], identity=id64[:])
            fragT = smp.tile([cfg.WPB, 64], f32, tag='masfragT')
            nc.scalar.activation(fragT[:], ps_t[:], AF.Identity)
            m_ps = psX.tile([P, cfg.GPB], f32, space="PSUM", tag='aux')
            for lb in range(cfg.LB):
                nc.tensor.matmul(
                    m_ps[:], lhsT=wm_sb[:, lb, :],
                    rhs=fragT[:, lb * cfg.GPB:(lb + 1) * cfg.GPB],
                    start=(lb == 0), stop=(lb == cfg.LB - 1))
            m_fm = smp.tile([P, cfg.GPB], f32, tag='masfm')
            nc.scalar.activation(m_fm[:], m_ps[:], AF.Identity,
                                 bias=bm_sb[:, 0:1])
            t_mas = cst.tile([P, cfg.B], f32, tag='tmas')
            nc.vector.tensor_tensor(
                out=t_mas[:].rearrange("p (s g) -> p s g", g=cfg.GPB),
                in0=m_fm[:, None, :].to_broadcast([P, NQ, cfg.GPB]),
                in1=gmask_sb[:, :].rearrange("p (s g) -> p s g", g=cfg.GPB),
                op=OP.mult)

            # ---------------- xw compute (two source halves)
            xwA = drp.tile([cfg.XWA_ROWS, cfg.F], fp8, tag='xwA')
            xwB = drp.tile([cfg.XWB_ROWS, cfg.F], fp8, tag='xwB')
            # bias row + zero row of xwA
            zrow = smp.tile([1, cfg.F], fp8, tag='zrow')
            nc.vector.memset(zrow[:], 0.0)
            nc.sync.dma_start(out=xwA[cfg.SH:cfg.SH + 1, :], in_=b8_sb[:])
            nc.sync.dma_start(out=xwA[cfg.SH + 1:cfg.SH + 2, :], in_=zrow[:])

            NSLAB = cfg.NPAD // 512               # 20 slabs of 512 nodes
            for sl in range(NSLAB):
                n0 = sl * 512
                xt_t = xtp.tile([P, cfg.KC2, 2, 512], fp8, tag='xt')
                for c in range(cfg.KC2):
                    for i in range(2):
                        nc.sync.dma_start(
                            out=xt_t[:, c, i, :],
                            in_=xt_d[c, :, i, n0:n0 + 512])
                for sub in range(4):
                    xw_t = xwsb.tile([P, cfg.F], fp8, tag='xwsb')
                    for fh in range(2):
                        ps = psA.tile([P, 512], f32, space="PSUM", tag='xwps')
                        for c in range(cfg.KC2):
                            if USE_DR:
                                nc.tensor.matmul(
                                    ps[:],
                                    lhsT=xt_t[:, c, :, sub * P:(sub + 1) * P],
                                    rhs=wg_sb[:, c, :, fh * 512:(fh + 1) * 512],
                                    start=(c == 0), stop=(c == cfg.KC2 - 1),
                                    perf_mode=DR)
                            else:
                                for i in range(2):
                                    nc.tensor.matmul(
                                        ps[:],
                                        lhsT=xt_t[:, c, i,
                                                  sub * P:(sub + 1) * P],
                                        rhs=wg_sb[:, c, i,
                                                  fh * 512:(fh + 1) * 512],
                                        start=(c == 0 and i == 0),
                                        stop=(c == cfg.KC2 - 1 and i == 1))
                        nc.scalar.activation(xw_t[:, fh * 512:(fh + 1) * 512],
                                             ps[:], AF.Identity, scale=0.125)
                    row = n0 + sub * P
                    if row < cfg.SH:
                        nc.sync.dma_start(out=xwA[row:row + P, :], in_=xw_t[:])
                    else:
                        nc.sync.dma_start(
                            out=xwB[row - cfg.SH:row - cfg.SH + P, :],
                            in_=xw_t[:])

            # ---------------- gather + scatter + pool
            pool_ps = [psP.tile([cfg.B, 512], f32, space="PSUM",
                                name=f'poolps{fh}') for fh in range(2)]
            blk_ps = {}
            gat_tiles = {}
            # per-group gathers; chunks reference their group's tile
            ch_by_grp = {}
            for ch in chunks:
                ch_by_grp.setdefault(ch['grp'], []).append(ch)

            pooled_n = [0]

            def finish_block(j, ps_pair, phase):
                if phase == 0:
                    # stage A partial (8x scale) into hA as fp8
                    for fh in range(2):
                        nc.scalar.activation(
                            hA[:, j * cfg.F + fh * 512:
                               j * cfg.F + (fh + 1) * 512],
                            ps_pair[fh][:], AF.Identity)
                else:
                    h_t = hp.tile([P, cfg.F], fp8, tag='h')
                    for fh in range(2):
                        nc.scalar.activation(
                            h_t[:, fh * 512:(fh + 1) * 512], ps_pair[fh][:],
                            AF.Lrelu, scale=0.125, alpha=0.01)
                    for fh in range(2):
                        nc.tensor.matmul(
                            pool_ps[fh][:],
                            lhsT=mpool_sb[:, j * cfg.B:(j + 1) * cfg.B],
                            rhs=h_t[:, fh * 512:(fh + 1) * 512],
                            start=(pooled_n[0] == 0),
                            stop=(pooled_n[0] == cfg.NBLK - 1))
                    pooled_n[0] += 1

            for g in range(n_groups):
                hf = groups[g][0]
                src = xwA if hf == 0 else xwB
                gat = gatp.tile([P, GRPU, cfg.F], fp8, tag='gat')
                nc.gpsimd.dma_gather(
                    out_ap=gat[:], in_ap=src[:, :],
                    idxs_ap=idx_sb[:, g * 64:(g + 1) * 64],
                    num_idxs=GRPU * P, num_idxs_reg=GRPU * P,
                    elem_size=cfg.F)
                for ch in ch_by_grp[g]:
                    j, u0 = ch['j'], ch['u0']
                    if ch['first']:
                        pair = [psB.tile([P, 512], f32, space="PSUM",
                                         name=f'blkps{fh}') for fh in range(2)]
                        blk_ps[(j, hf)] = pair
                        if hf == 1:
                            for fh in range(2):
                                nc.tensor.matmul(
                                    pair[fh][:], lhsT=id128[:],
                                    rhs=hA[:, j * cfg.F + fh * 512:
                                           j * cfg.F + (fh + 1) * 512],
                                    start=True, stop=False)
                    pair = blk_ps[(j, hf)]
                    sm0 = (g * GRPU + u0) * P
                    st = ch['first'] and hf == 0
                    sp = ch['last']
                    for fh in range(2):
                        if ch['units'] == 2 and USE_DR:
                            nc.tensor.matmul(
                                pair[fh][:],
                                lhsT=smat_sb[:, sm0:sm0 + 2 * P].rearrange(
                                    "p (i d) -> p i d", i=2),
                                rhs=gat[:, u0:u0 + 2,
                                        fh * 512:(fh + 1) * 512],
                                start=st, stop=sp, perf_mode=DR)
                        else:
                            for i in range(ch['units']):
                                nc.tensor.matmul(
                                    pair[fh][:],
                                    lhsT=smat_sb[:, sm0 + i * P:
                                                 sm0 + (i + 1) * P],
                                    rhs=gat[:, u0 + i,
                                            fh * 512:(fh + 1) * 512],
                                    start=(st and i == 0),
                                    stop=(sp and i == ch['units'] - 1))
                for ch in ch_by_grp[g]:
                    if ch['last']:
                        finish_block(ch['j'], blk_ps.pop((ch['j'], hf)), hf)

            # ---------------- pooled -> x1 partial
            pooled_sb = smp.tile([cfg.B, cfg.F], f32, tag='pooled')
            for fh in range(2):
                nc.scalar.activation(pooled_sb[:, fh * 512:(fh + 1) * 512],
                                     pool_ps[fh][:], AF.Identity,
                                     scale=float(2.0 ** -8))
            pfm = smp.tile([P, 8, cfg.B], f32, tag='pfm')
            for k in range(8):
                tps = psX.tile([P, cfg.B], f32, space="PSUM", tag='aux')
                nc.tensor.transpose(
                    out=tps[:], in_=pooled_sb[:, k * P:(k + 1) * P],
                    identity=id32[:])
                nc.scalar.activation(pfm[:, k, :], tps[:], AF.Identity)
            xps = psX.tile([P, cfg.B], f32, space="PSUM", tag='aux')
            for k in range(8):
                nc.tensor.matmul(xps[:], lhsT=wpf_sb[:, k, :],
                                 rhs=pfm[:, k, :],
                                 start=(k == 0), stop=(k == 7))
            x1p = smp.tile([P, cfg.B], f32, tag='x1p')
            nc.scalar.activation(x1p[:], xps[:], AF.Identity)

            # ---------------- cc packing + allreduce
            t_x1 = smp.tile([P, cfg.B], f32, tag='tx1')
            t_x2 = smp.tile([P, cfg.B], f32, tag='tx2')
            nc.scalar.activation(t_x1[:], x1p[:], AF.Identity,
                                 scale=mask1_sb[:, 0:1])
            nc.scalar.activation(t_x2[:], x1p[:], AF.Identity,
                                 scale=mask2_sb[:, 0:1])
            bounce_in = drp.tile([CC], f32, tag='ccin')
            bounce_out = drp.tile([CC], f32, tag='ccout')
            seg = P * cfg.B
            for i, t in enumerate((t_x1, t_x2, t_mas)):
                nc.sync.dma_start(
                    out=bounce_in[i * seg:(i + 1) * seg].rearrange(
                        "(p f) -> p f", f=cfg.B),
                    in_=t[:])
            nc.gpsimd.collective_compute(
                "AllReduce", OP.add,
                replica_groups=[list(range(N_CORES))],
                ins=[bounce_in[:].opt()], outs=[bounce_out[:].opt()])

            x12 = {}
            for brr, bpf in ((1, bpf1_sb), (2, bpf2_sb)):
                xs = smp.tile([P, cfg.B], f32, tag=f'x{brr}')
                nc.sync.dma_start(
                    out=xs[:],
                    in_=bounce_out[(brr - 1) * seg:brr * seg].rearrange(
                        "(p f) -> p f", f=cfg.B))
                nc.scalar.activation(xs[:], xs[:], AF.Lrelu,
                                     bias=bpf[:, 0:1], alpha=0.01)
                x12[brr] = xs
            masif_rb = smp.tile([P, cfg.B], f32, tag='masrb')
            nc.sync.dma_start(
                out=masif_rb[:],
                in_=bounce_out[2 * seg:3 * seg].rearrange(
                    "(p f) -> p f", f=cfg.B))

            # ---------------- head
            xc1 = {}
            for mh in range(2):
                cps = psX.tile([P, cfg.B], f32, space="PSUM", tag='aux')
                for k2 in range(2):
                    nc.tensor.matmul(
                        cps[:], lhsT=wfc1_sb[:, k2, mh * P:(mh + 1) * P],
                        rhs=x12[k2 + 1][:], start=(k2 == 0), stop=(k2 == 1))
                xcs = smp.tile([P, cfg.B], f32, tag=f'xc{mh}')
                nc.scalar.activation(xcs[:], cps[:], AF.Lrelu,
                                     bias=bfc1_sb[:, mh, 0:1], alpha=0.01)
                xc1[mh] = xcs
            c2ps = psX.tile([64, cfg.B], f32, space="PSUM", tag='aux')
            for k2 in range(2):
                nc.tensor.matmul(c2ps[:], lhsT=wfc2_sb[:, k2, :],
                                 rhs=xc1[k2][:], start=(k2 == 0),
                                 stop=(k2 == 1))
            xc = smp.tile([64, cfg.B], f32, tag='xcf')
            nc.scalar.activation(xc[:], c2ps[:], AF.Lrelu,
                                 bias=bfc2_sb[:, 0:1], alpha=0.01)

            ops = psX.tile([1, cfg.B], f32, space="PSUM", tag='aux')
            nc.tensor.matmul(ops[:], lhsT=wout_x_sb[:], rhs=xc[:],
                             start=True, stop=False)
            nc.tensor.matmul(ops[:], lhsT=wout_m_sb[:], rhs=masif_rb[:],
                             start=False, stop=True)
            res = smp.tile([1, cfg.B], f32, tag='res')
            nc.scalar.activation(res[:], ops[:], AF.Sigmoid,
                                 bias=bout_sb[:, 0:1])
            nc.sync.dma_start(out=out_t[:, :], in_=res[:])

    nc.compile()
    return nc


# ---------------------------------------------------------------- entry
_CACHE = {}


def _run(inputs, cfg, trace=False, tmpdir=None):
    from concourse import bass_utils
    meta, in_maps = _preprocess(inputs, cfg)
    key = tuple((c['hf'], c['j'], c['units'], c['u0'], c['grp'],
                 c['first'], c['last']) for c in meta['chunks'])
    if key not in _CACHE:
        _CACHE.clear()
        _CACHE[key] = _build(cfg, meta)
    nc = _CACHE[key]
    res = bass_utils.run_bass_kernel_spmd(
        nc, in_maps, core_ids=list(range(N_CORES)), trace=trace, tmpdir=tmpdir)
    out = np.asarray(res.results[0]['out'], np.float32).reshape(cfg.B, 1)
    return out, res


def kernel(**inputs) -> np.ndarray:
    cfg = _Cfg()
    out, _ = _run(inputs, cfg)
    return out


# revision 18
# speedup vs baseline: 1.7841x; 1.1070x over previous
"""Trainium2 Bass kernel for nn_GCNN_87668872446200.

Branch-split design over 8 NeuronCores: cores 0-3 run protein branch 1,
cores 4-7 run branch 2.  Within a branch group each core owns a quarter of
the destination nodes and the full F=1024 feature dim.

Per core (fp8 e4m3 data paths, DoubleRow fp8 matmuls):
  - xw' = 8*(x*dinv_row) @ (W*64) / 8   computed on PE in two source-halves,
    written to HBM as two tensors (xwA rows <5120 + bias row, xwB rest)
  - symmetric norm is separated: h = Dinv A Dinv xw + b realized as
    S-matmul with S[e,d] = dinv[d] (bias via a virtual edge to a bias row)
  - dma_gather pulls 1KB fp8 rows per edge; source-half split lets the
    Q7 descriptor emission of half A overlap the xw compute of half B
  - phase A partial sums staged in SBUF (fp8), injected into phase B PSUM
    via an identity matmul; one ACT pass does lrelu(psum/8)
  - per-graph mean-pool as PE matmul (mpool*256 fp8), W_pf applied locally
  - masif branch (8 graphs/core, this core's branch only)
  - one small AllReduce ([3,128,32] f32 = 48KB) + replicated dense head

All 8 cores run ONE identical program; per-core variation is in input data.
"""
import numpy as np

# ---------------------------------------------------------------- constants
N_CORES = 8
P = 128
BLK = 128           # dest nodes per block
NQ = 4              # dest quarters per branch group
GRPU = 8            # 128-idx units per gather call (1024 idxs)

N_NODES, N_EDGES, F_DIM, B_GRAPHS, L_MAS, C_MAS = 10000, 80000, 1024, 32, 800, 16

USE_DR = True       # DoubleRow fp8 matmuls


class _Cfg:
    def __init__(self, n=N_NODES, e=N_EDGES, f=F_DIM, b=B_GRAPHS,
                 l=L_MAS, c=C_MAS):
        self.N, self.E, self.F, self.B, self.L, self.C = n, e, f, b, l, c
        self.NPAD = ((n + 511) // 512) * 512          # 10240
        self.QH = self.NPAD // NQ                     # 2560 dests per core
        self.NBLK = self.QH // BLK                    # 20 blocks
        self.SH = self.NPAD // 2                      # 5120 source-half split
        self.KC2 = f // 256                           # 4 k-pairs
        self.GPB = b // 4                             # 8 graphs per core
        self.LW = l // 80                             # 10
        self.LB = 8                                   # l-blocks
        self.LBS = l // self.LB                       # 100
        self.WPB = self.LBS // self.LW                # 10
        # xwA holds source rows 0..SH-1 plus bias row (SH) and zero row (SH+1)
        self.XWA_ROWS = self.SH + P                   # 5248
        self.XWB_ROWS = self.NPAD - self.SH           # 5120 (tail rows zero)


def _q8(x):
    import ml_dtypes
    return np.clip(np.asarray(x, np.float32), -240.0, 240.0).astype(
        ml_dtypes.float8_e4m3)


# ---------------------------------------------------------------- host prep
def _edge_plan_core(cfg, edge_index, q):
    """Edges targeting quarter q, split per (block, source-half), sorted.
    Returns dict (j, hf) -> (rows, dests, counts)."""
    row = np.asarray(edge_index[0]).astype(np.int64)
    col = np.asarray(edge_index[1]).astype(np.int64)
    loops = np.arange(cfg.N, dtype=np.int64)
    rows = np.concatenate([row, loops])
    cols = np.concatenate([col, loops])
    lo, hi = q * cfg.QH, (q + 1) * cfg.QH
    sel = (cols >= lo) & (cols < hi)
    r, c = rows[sel], cols[sel] - lo
    out = {}
    for j in range(cfg.NBLK):
        bsel = (c >= j * BLK) & (c < (j + 1) * BLK)
        rj, cj = r[bsel], c[bsel] - j * BLK
        for hf in range(2):
            hsel = (rj < cfg.SH) if hf == 0 else (rj >= cfg.SH)
            out[(j, hf)] = (rj[hsel], cj[hsel])
    return out


def _shared_schedule(cfg, plans):
    """Shared chunk schedule (max over the 8 per-core plans).

    Returns chunks: list of dicts with keys
      hf, j, units (1 or 2), u0 (unit offset in group), grp (group index),
      first (starts block), last (ends block's half... block completion is
      tracked at (j,hf==1,last) for B and (j,hf==0,last) for A)
    and n_groups_a / n_groups_b.
    """
    chunks = []
    groups = []                                   # list of [hf, nunits]
    for hf in range(2):
        space = 0                                 # force new group per half
        for j in range(cfg.NBLK):
            e_max = max(len(p[(j, hf)][0]) for p in plans)
            if hf == 0:
                e_max += 1                        # bias slot
            left = max(1, (e_max + P - 1) // P)   # units needed
            first = True
            while left:
                if space == 0:
                    groups.append([hf, 0])
                    space = GRPU
                sz = 2 if (left >= 2 and space >= 2) else 1
                chunks.append(dict(hf=hf, j=j, units=sz, u0=GRPU - space,
                                   grp=len(groups) - 1,
                                   first=first, last=(left - sz == 0)))
                space -= sz
                left -= sz
                first = False
                groups[-1][1] = GRPU - space
    return chunks, groups


def _fill_core_gather2(cfg, chunks, groups, plan, dinv, q):
    """Per-core idx + smat content for the shared schedule (correct
    multi-chunk consumption)."""
    n_groups = len(groups)
    flat_idx = np.zeros((n_groups, GRPU * P), np.int64)
    smat = np.zeros((n_groups, P, GRPU * P), np.float32)
    dinv8 = _q8(dinv).astype(np.float32)
    consumed = {}
    for ch in chunks:
        j, hf, g, u0 = ch['j'], ch['hf'], ch['grp'], ch['u0']
        r, c = plan[(j, hf)]
        off = consumed.get((j, hf), 0)
        cap = ch['units'] * P
        base = u0 * P
        pad_idx = cfg.SH + 1 if hf == 0 else cfg.XWB_ROWS - 1
        flat_idx[g, base:base + cap] = pad_idx
        s = 0
        if hf == 0 and ch['first']:
            flat_idx[g, base] = cfg.SH            # bias row at slot 0
            smat[g, 0, base:base + P] = 1.0
            s = 1
        take = min(len(r) - off, cap - s)
        if take > 0:
            rr = r[off:off + take]
            cc = c[off:off + take]
            if hf == 1:
                rr = rr - cfg.SH
            slots = np.arange(s, s + take) + base
            up = slots // P
            pp = slots % P
            flat_idx[g, slots] = rr
            # dest scale dinv[global dest] ; global dest = q*QH + j*BLK + cc
            gd = q * cfg.QH + j * BLK + cc
            vals = dinv8[np.minimum(gd, cfg.N - 1)] * (gd < cfg.N)
            smat[g, pp, up * P + cc] = vals
        consumed[(j, hf)] = off + take
    for (j, hf), off in consumed.items():
        assert off == len(plan[(j, hf)][0]), (j, hf, off, len(plan[(j, hf)][0]))
    return flat_idx, smat


def _wrap_idx_groups(flat_idx):
    """[G, 1024] -> [128, G*64] int16 (16-part wrap, 8x replicated)."""
    g, n = flat_idx.shape
    w = flat_idx.reshape(g, n // 16, 16).transpose(2, 0, 1).reshape(16, -1)
    return np.tile(w, (8, 1)).astype(np.int16)


def _preprocess(inputs, cfg):
    import ml_dtypes
    bf16 = ml_dtypes.bfloat16
    f32 = np.float32

    # --- per-branch shared data
    bdata = {}
    for br in (1, 2):
        x = np.asarray(inputs[f'pro{br}_x'], f32)
        ei = np.asarray(inputs[f'pro{br}_edge_index'])
        batch = np.asarray(inputs[f'pro{br}_batch']).astype(np.int64)
        row = ei[0].astype(np.int64)
        col = ei[1].astype(np.int64)
        deg = np.bincount(np.concatenate([col, np.arange(cfg.N)]),
                          minlength=cfg.N).astype(np.float64)
        dinv = (1.0 / np.sqrt(deg)).astype(f32)
        # xt_dr [NSLAB, 128, KC2*2*512] fp8 of (x*dinv_row)^T, slab-major
        xp = x * dinv[:, None]
        xpT = np.zeros((cfg.F, cfg.NPAD), f32)
        xpT[:, :cfg.N] = xp.T
        nslab = cfg.NPAD // 512
        xt_dr = np.ascontiguousarray(
            xpT.reshape(cfg.KC2, 2, P, nslab, 512).transpose(3, 2, 0, 1, 4)
        ).reshape(nslab, P, cfg.KC2 * 2 * 512)
        # wg_dr [128, KC2*2*F] fp8  (p, c, i, f)
        W = np.asarray(inputs[f'W_g{br}'], f32) * 64.0
        wg_dr = np.ascontiguousarray(
            W.reshape(cfg.KC2, 2, P, cfg.F).transpose(2, 0, 1, 3)
        ).reshape(P, cfg.KC2 * 2 * cfg.F)
        b8 = np.asarray(inputs[f'b_g{br}'], f32) * 8.0
        cnt = np.bincount(batch, minlength=cfg.B).astype(f32)
        plans = [_edge_plan_core(cfg, ei, q) for q in range(NQ)]
        bdata[br] = dict(xt=_q8(xt_dr), wg=_q8(wg_dr), b8=_q8(b8[None, :]),
                         dinv=dinv, batch=batch, cnt=cnt, plans=plans)

    # --- shared chunk schedule (max over all 8 core plans)
    all_plans = bdata[1]['plans'] + bdata[2]['plans']
    chunks, groups = _shared_schedule(cfg, all_plans)
    n_groups = len(groups)
    n_ga = sum(1 for g in groups if g[0] == 0)

    meta = dict(chunks=chunks, groups=groups, n_groups=n_groups, n_ga=n_ga)

    # --- head weights (shared across cores)
    shared = {}
    shared['wfc1'] = np.ascontiguousarray(
        np.asarray(inputs['W_fc1'], f32).reshape(2, P, 256).transpose(1, 0, 2))
    shared['wfc2'] = np.ascontiguousarray(
        np.asarray(inputs['W_fc2'], f32).reshape(2, P, 64).transpose(1, 0, 2))
    shared['bfc1'] = np.ascontiguousarray(
        np.asarray(inputs['b_fc1'], f32).reshape(2, P, 1).transpose(1, 0, 2))
    shared['bfc2'] = np.asarray(inputs['b_fc2'], f32).reshape(64, 1)
    wout = np.asarray(inputs['W_out'], f32)
    shared['wout_x'] = np.ascontiguousarray(wout[0:64])            # [64,1]
    shared['wout_m'] = np.ascontiguousarray(wout[64:192])          # [128,1]
    shared['bout'] = np.asarray(inputs['b_out'], f32).reshape(1, 1)
    shared['bpf1'] = np.asarray(inputs['b_pf1'], f32).reshape(P, 1)
    shared['bpf2'] = np.asarray(inputs['b_pf2'], f32).reshape(P, 1)
    shared['id32'] = np.eye(32, dtype=f32)
    shared['id128_8'] = _q8(np.eye(P, dtype=f32))
    shared['id64'] = np.eye(64, dtype=f32)

    in_maps = []
    for core in range(N_CORES):
        br = 1 + core // NQ
        q = core % NQ
        bd = bdata[br]
        m = dict(shared)
        m['xt'] = bd['xt']
        m['wg'] = bd['wg']
        m['b8row'] = bd['b8']
        # gather plan
        flat_idx, smat = _fill_core_gather2(
            cfg, chunks, groups, bd['plans'][q], bd['dinv'], q)
        m['idx'] = _wrap_idx_groups(flat_idx)
        m['smat'] = np.ascontiguousarray(
            smat.transpose(1, 0, 2).reshape(P, n_groups * GRPU * P)).astype(
            ml_dtypes.float8_e4m3)
        # mpool [128, NBLK, B] fp8 (x256)
        mp = np.zeros((P, cfg.NBLK, cfg.B), f32)
        for j in range(cfg.NBLK):
            nodes = q * cfg.QH + j * BLK + np.arange(BLK)
            ok = nodes < cfg.N
            gidx = bd['batch'][np.minimum(nodes, cfg.N - 1)]
            val = 256.0 / np.maximum(bd['cnt'][gidx], 1.0) * ok
            mp[np.arange(BLK), j, gidx] = val
        m['mpool'] = _q8(mp.reshape(P, cfg.NBLK * cfg.B))
        # W_pf for this branch  [128, 8*128] f32  (p, k, m)
        wpf = np.asarray(inputs[f'W_pf{br}'], f32)
        m['wpf'] = np.ascontiguousarray(
            wpf.reshape(8, P, P).transpose(1, 0, 2)).reshape(P, 8 * P)
        # branch masks for cc packing
        m['mask1'] = np.full((P, 1), 1.0 if br == 1 else 0.0, f32)
        m['mask2'] = np.full((P, 1), 1.0 if br == 2 else 0.0, f32)
        # masif (this branch only, 8 graphs) laid out [64=(lb,g), C*LBS]
        gs = (core % NQ) * cfg.GPB
        for sfk, name in (('s', 'straight'), ('f', 'flipped')):
            src = np.asarray(inputs[f'mas{br}_{name}'], f32)[gs:gs + cfg.GPB]
            # [g, ch, lb*LBS+l] -> [(lb, g), ch, l]
            r = src.reshape(cfg.GPB, cfg.C, cfg.LB, cfg.LBS).transpose(
                2, 0, 1, 3)
            m[f'mas_{sfk}'] = np.ascontiguousarray(r).reshape(
                64, cfg.C * cfg.LBS)
        # wm128 [10, 8, 128]: this branch's W_m/(2*LW) at rows 64*(br-1)..
        wm = np.zeros((cfg.WPB, cfg.LB, P), f32)
        wsrc = (np.asarray(inputs[f'W_m{br}'], f32) / (2.0 * cfg.LW)).reshape(
            cfg.LB, cfg.WPB, 64)
        wm[:, :, 64 * (br - 1):64 * br] = wsrc.transpose(1, 0, 2)
        m['wm'] = np.ascontiguousarray(wm)
        bm = np.zeros((P, 1), f32)
        bm[64 * (br - 1):64 * br, 0] = np.asarray(inputs[f'b_m{br}'], f32)
        m['bm'] = bm
        gm = np.zeros((P, cfg.B), f32)
        gm[64 * (br - 1):64 * br, gs:gs + cfg.GPB] = 1.0
        m['gmask'] = gm
        for sf, pre in (('s', 'cs'), ('f', 'cf')):
            w = float(np.asarray(inputs[f'{pre}{br}_w'])[0])
            b = float(np.asarray(inputs[f'{pre}{br}_b'])[0])
            m[f'scale_{sf}'] = np.full((64, 1), w / cfg.C, f32)
            m[f'bias_{sf}'] = np.full((64, 1), b, f32)
        in_maps.append(m)
    return meta, in_maps


# ---------------------------------------------------------------- program
def _build(cfg, meta):
    import concourse.bass as bass
    import concourse.bacc as bacc
    import concourse.mybir as mybir
    import concourse.tile as tile

    dt = mybir.dt
    f32 = dt.float32
    fp8 = dt.float8e4
    AF = mybir.ActivationFunctionType
    OP = mybir.AluOpType
    DR = mybir.MatmulPerfMode.DoubleRow if USE_DR else None

    chunks = meta['chunks']
    groups = meta['groups']
    n_groups = meta['n_groups']
    n_ga = meta['n_ga']

    nc = bacc.Bacc("TRN2", target_bir_lowering=False, debug=False,
                   enable_asserts=False, num_devices=N_CORES)

    def din(name, shape, d):
        return nc.dram_tensor(name, list(shape), d, kind="ExternalInput")

    NSLAB = cfg.NPAD // 512
    xt_d = din('xt', (NSLAB, P, cfg.KC2 * 2 * 512), fp8)
    wg_d = din('wg', (P, cfg.KC2 * 2 * cfg.F), fp8)
    b8_d = din('b8row', (1, cfg.F), fp8)
    idx_d = din('idx', (P, n_groups * 64), dt.int16)
    smat_d = din('smat', (P, n_groups * GRPU * P), fp8)
    mpool_d = din('mpool', (P, cfg.NBLK * cfg.B), fp8)
    wpf_d = din('wpf', (P, 8 * P), f32)
    mas_d = {sf: din(f'mas_{sf}', (64, cfg.C * cfg.LBS), f32) for sf in 'sf'}
    wm_d = din('wm', (cfg.WPB, cfg.LB, P), f32)
    bm_d = din('bm', (P, 1), f32)
    gmask_d = din('gmask', (P, cfg.B), f32)
    msc_d = {(sf, kind): din(f'{kind}_{sf}', (64, 1), f32)
             for sf in 'sf' for kind in ('scale', 'bias')}
    wfc1_d = din('wfc1', (P, 2, 256), f32)
    wfc2_d = din('wfc2', (P, 2, 64), f32)
    bfc1_d = din('bfc1', (P, 2, 1), f32)
    bfc2_d = din('bfc2', (64, 1), f32)
    wout_x_d = din('wout_x', (64, 1), f32)
    wout_m_d = din('wout_m', (P, 1), f32)
    bout_d = din('bout', (1, 1), f32)
    bpf1_d = din('bpf1', (P, 1), f32)
    bpf2_d = din('bpf2', (P, 1), f32)
    mask1_d = din('mask1', (P, 1), f32)
    mask2_d = din('mask2', (P, 1), f32)
    id32_d = din('id32', (32, 32), f32)
    id64_d = din('id64', (64, 64), f32)
    id128_d = din('id128_8', (P, P), fp8)

    out_t = nc.dram_tensor('out', [1, cfg.B], f32, kind="ExternalOutput")

    CC = 3 * P * cfg.B          # allreduce payload (f32 elements)

    with tile.TileContext(nc) as tc:
        with tc.tile_pool(name="const", bufs=1) as cst, \
             tc.tile_pool(name="xt", bufs=2) as xtp, \
             tc.tile_pool(name="xwsb", bufs=3) as xwsb, \
             tc.tile_pool(name="gat", bufs=5) as gatp, \
             tc.tile_pool(name="hsb", bufs=2) as hp, \
             tc.tile_pool(name="small", bufs=2) as smp, \
             tc.tile_pool(name="psA", bufs=2, space="PSUM") as psA, \
             tc.tile_pool(name="psBlk", bufs=1, space="PSUM") as psB, \
             tc.tile_pool(name="psPool", bufs=1, space="PSUM") as psP, \
             tc.tile_pool(name="psX", bufs=2, space="PSUM") as psX, \
             tc.tile_pool(name="dram", bufs=1, space="DRAM") as drp:

            def load(pool, src_ap, shape, d, name=None):
                t = pool.tile(list(shape), d, tag=name)
                nc.sync.dma_start(out=t[:], in_=src_ap)
                return t

            # ---------------- xw-critical constants first
            wg_flat = load(cst, wg_d[:, :], (P, cfg.KC2 * 2 * cfg.F), fp8,
                           'wg')
            wg_sb = wg_flat[:].rearrange("p (c i f) -> p c i f", c=cfg.KC2,
                                         i=2)

            # ---------------- xw compute: A half (source rows < SH)
            xwA = drp.tile([cfg.XWA_ROWS, cfg.F], fp8, tag='xwA')
            xwB = drp.tile([cfg.XWB_ROWS, cfg.F], fp8, tag='xwB')

            def xw_slab(sl):
                n0 = sl * 512
                xt_flat = xtp.tile([P, cfg.KC2 * 2 * 512], fp8, tag='xt',
                                   name='xt_t')
                nc.sync.dma_start(out=xt_flat[:], in_=xt_d[sl, :, :])
                xt_t = xt_flat[:].rearrange("p (c i n) -> p c i n", c=cfg.KC2,
                                            i=2)
                for sub in range(4):
                    xw_t = xwsb.tile([P, cfg.F], fp8, tag='xwsb', name='xw_t')
                    for fh in range(2):
                        ps = psA.tile([P, 512], f32, space="PSUM", tag='xwps',
                                      name='xw_ps')
                        for c in range(cfg.KC2):
                            if USE_DR:
                                nc.tensor.matmul(
                                    ps[:],
                                    lhsT=xt_t[:, c, :, sub * P:(sub + 1) * P],
                                    rhs=wg_sb[:, c, :, fh * 512:(fh + 1) * 512],
                                    start=(c == 0), stop=(c == cfg.KC2 - 1),
                                    perf_mode=DR)
                            else:
                                for i in range(2):
                                    nc.tensor.matmul(
                                        ps[:],
                                        lhsT=xt_t[:, c, i,
                                                  sub * P:(sub + 1) * P],
                                        rhs=wg_sb[:, c, i,
                                                  fh * 512:(fh + 1) * 512],
                                        start=(c == 0 and i == 0),
                                        stop=(c == cfg.KC2 - 1 and i == 1))
                        nc.scalar.activation(xw_t[:, fh * 512:(fh + 1) * 512],
                                             ps[:], AF.Identity, scale=0.125)
                    row = n0 + sub * P
                    if row < cfg.SH:
                        nc.sync.dma_start(out=xwA[row:row + P, :], in_=xw_t[:])
                    else:
                        nc.sync.dma_start(
                            out=xwB[row - cfg.SH:row - cfg.SH + P, :],
                            in_=xw_t[:])

            for sl in range(NSLAB // 2):
                xw_slab(sl)

            # ---------------- remaining constants (overlap with xw PE)
            idx_sb = load(cst, idx_d[:, :], (P, n_groups * 64), dt.int16, 'idx')
            smat_sb = load(cst, smat_d[:, :], (P, n_groups * GRPU * P), fp8,
                           'smat')
            mpool_sb = load(cst, mpool_d[:, :], (P, cfg.NBLK * cfg.B), fp8,
                            'mpool')
            wpf_flat = load(cst, wpf_d[:, :], (P, 8 * P), f32, 'wpf')
            wpf_sb = wpf_flat[:].rearrange("p (k m) -> p k m", k=8)
            wm_sb = load(cst, wm_d[:, :, :], (cfg.WPB, cfg.LB, P), f32, 'wm')
            bm_sb = load(cst, bm_d[:, :], (P, 1), f32, 'bm')
            gmask_sb = load(cst, gmask_d[:, :], (P, cfg.B), f32, 'gmask')
            msc_sb = {k: load(cst, v[:, :], (64, 1), f32, f'msc{k}')
                      for k, v in msc_d.items()}
            wfc1_sb = load(cst, wfc1_d[:, :, :], (P, 2, 256), f32, 'wfc1')
            wfc2_sb = load(cst, wfc2_d[:, :, :], (P, 2, 64), f32, 'wfc2')
            bfc1_sb = load(cst, bfc1_d[:, :, :], (P, 2, 1), f32, 'bfc1')
            bfc2_sb = load(cst, bfc2_d[:, :], (64, 1), f32, 'bfc2')
            wout_x_sb = load(cst, wout_x_d[:, :], (64, 1), f32, 'woutx')
            wout_m_sb = load(cst, wout_m_d[:, :], (P, 1), f32, 'woutm')
            bout_sb = load(cst, bout_d[:, :], (1, 1), f32, 'bout')
            bpf1_sb = load(cst, bpf1_d[:, :], (P, 1), f32, 'bpf1')
            bpf2_sb = load(cst, bpf2_d[:, :], (P, 1), f32, 'bpf2')
            mask1_sb = load(cst, mask1_d[:, :], (P, 1), f32, 'mask1')
            mask2_sb = load(cst, mask2_d[:, :], (P, 1), f32, 'mask2')
            id32 = load(cst, id32_d[:, :], (32, 32), f32, 'id32')
            id64 = load(cst, id64_d[:, :], (64, 64), f32, 'id64')
            id128 = load(cst, id128_d[:, :], (P, P), fp8, 'id128')
            b8_sb = load(cst, b8_d[:, :], (1, cfg.F), fp8, 'b8')

            hA = cst.tile([P, cfg.NBLK * cfg.F], fp8, tag='hA')

            # ---------------- masif (one branch, 8 graphs -> [128, B] via PE)
            frag = None
            for sf in 'sf':
                tf = smp.tile([64, cfg.C * cfg.LBS], f32, tag='masload',
                              name='mas_t')
                nc.sync.dma_start(out=tf[:], in_=mas_d[sf][:, :])
                t = tf[:].rearrange("p (c l) -> p c l", c=cfg.C)
                red = smp.tile([64, cfg.LBS], f32, tag='masred')
                nc.vector.tensor_reduce(
                    out=red[:], in_=t.transpose([0, 2, 1]),
                    axis=mybir.AxisListType.X, op=OP.add)
                act = smp.tile([64, cfg.LBS], f32, tag='masact')
                nc.scalar.activation(
                    act[:], red[:], AF.Relu,
                    bias=msc_sb[(sf, 'bias')][:, 0:1],
                    scale=msc_sb[(sf, 'scale')][:, 0:1])
                ws = smp.tile([64, cfg.WPB], f32, tag='masws')
                nc.vector.tensor_reduce(
                    out=ws[:],
                    in_=act[:].rearrange("p (w l) -> p w l", l=cfg.LW),
                    axis=mybir.AxisListType.X, op=OP.add)
                if frag is None:
                    frag = ws
                else:
                    frag2 = smp.tile([64, cfg.WPB], f32, tag='masfrag')
                    nc.vector.tensor_add(out=frag2[:], in0=frag[:], in1=ws[:])
                    frag = frag2
            ps_t = psX.tile([cfg.WPB, 64], f32, space="PSUM", tag='aux')
            nc.tensor.transpose(out=ps_t[:], in_=frag[:], identity=id64[:])
            fragT = smp.tile([cfg.WPB, 64], f32, tag='masfragT')
            nc.scalar.activation(fragT[:], ps_t[:], AF.Identity)
            m_ps = psX.tile([P, cfg.GPB], f32, space="PSUM", tag='aux')
            for lb in range(cfg.LB):
                nc.tensor.matmul(
                    m_ps[:], lhsT=wm_sb[:, lb, :],
                    rhs=fragT[:, lb * cfg.GPB:(lb + 1) * cfg.GPB],
                    start=(lb == 0), stop=(lb == cfg.LB - 1))
            m_fm = smp.tile([P, cfg.GPB], f32, tag='masfm')
            nc.scalar.activation(m_fm[:], m_ps[:], AF.Identity,
                                 bias=bm_sb[:, 0:1])
            t_mas = cst.tile([P, cfg.B], f32, tag='tmas')
            nc.vector.tensor_tensor(
                out=t_mas[:].rearrange("p (s g) -> p s g", g=cfg.GPB),
                in0=m_fm[:, None, :].to_broadcast([P, NQ, cfg.GPB]),
                in1=gmask_sb[:, :].rearrange("p (s g) -> p s g", g=cfg.GPB),
                op=OP.mult)

            # bias row + zero row of xwA, then B-half slabs
            zrow = smp.tile([1, cfg.F], fp8, tag='zrow')
            nc.vector.memset(zrow[:], 0.0)
            nc.sync.dma_start(out=xwA[cfg.SH:cfg.SH + 1, :], in_=b8_sb[:])
            nc.sync.dma_start(out=xwA[cfg.SH + 1:cfg.SH + 2, :], in_=zrow[:])

            for sl in range(NSLAB // 2, NSLAB):
                xw_slab(sl)

            # ---------------- gather + scatter + pool
            pool_ps = [psP.tile([cfg.B, 512], f32, space="PSUM",
                                name=f'poolps{fh}') for fh in range(2)]
            blk_ps = {}
            gat_tiles = {}
            # per-group gathers; chunks reference their group's tile
            ch_by_grp = {}
            for ch in chunks:
                ch_by_grp.setdefault(ch['grp'], []).append(ch)

            pooled_n = [0]

            def finish_block(j, ps_pair, phase):
                if phase == 0:
                    # stage A partial (8x scale) into hA as fp8
                    for fh in range(2):
                        nc.scalar.activation(
                            hA[:, j * cfg.F + fh * 512:
                               j * cfg.F + (fh + 1) * 512],
                            ps_pair[fh][:], AF.Identity)
                else:
                    h_t = hp.tile([P, cfg.F], fp8, tag='h')
                    for fh in range(2):
                        nc.scalar.activation(
                            h_t[:, fh * 512:(fh + 1) * 512], ps_pair[fh][:],
                            AF.Lrelu, scale=0.125, alpha=0.01)
                    for fh in range(2):
                        nc.tensor.matmul(
                            pool_ps[fh][:],
                            lhsT=mpool_sb[:, j * cfg.B:(j + 1) * cfg.B],
                            rhs=h_t[:, fh * 512:(fh + 1) * 512],
                            start=(pooled_n[0] == 0),
                            stop=(pooled_n[0] == cfg.NBLK - 1))
                    pooled_n[0] += 1

            for g in range(n_groups):
                hf = groups[g][0]
                src = xwA if hf == 0 else xwB
                gat = gatp.tile([P, GRPU, cfg.F], fp8, tag='gat')
                nc.gpsimd.dma_gather(
                    out_ap=gat[:], in_ap=src[:, :],
                    idxs_ap=idx_sb[:, g * 64:(g + 1) * 64],
                    num_idxs=GRPU * P, num_idxs_reg=GRPU * P,
                    elem_size=cfg.F)
                for ch in ch_by_grp[g]:
                    j, u0 = ch['j'], ch['u0']
                    if ch['first']:
                        pair = [psB.tile([P, 512], f32, space="PSUM",
                                         name=f'blkps{fh}') for fh in range(2)]
                        blk_ps[(j, hf)] = pair
                        if hf == 1:
                            for fh in range(2):
                                nc.tensor.matmul(
                                    pair[fh][:], lhsT=id128[:],
                                    rhs=hA[:, j * cfg.F + fh * 512:
                                           j * cfg.F + (fh + 1) * 512],
                                    start=True, stop=False)
                    pair = blk_ps[(j, hf)]
                    sm0 = (g * GRPU + u0) * P
                    st = ch['first'] and hf == 0
                    sp = ch['last']
                    for fh in range(2):
                        if ch['units'] == 2 and USE_DR:
                            nc.tensor.matmul(
                                pair[fh][:],
                                lhsT=smat_sb[:, sm0:sm0 + 2 * P].rearrange(
                                    "p (i d) -> p i d", i=2),
                                rhs=gat[:, u0:u0 + 2,
                                        fh * 512:(fh + 1) * 512],
                                start=st, stop=sp, perf_mode=DR)
                        else:
                            for i in range(ch['units']):
                                nc.tensor.matmul(
                                    pair[fh][:],
                                    lhsT=smat_sb[:, sm0 + i * P:
                                                 sm0 + (i + 1) * P],
                                    rhs=gat[:, u0 + i,
                                            fh * 512:(fh + 1) * 512],
                                    start=(st and i == 0),
                                    stop=(sp and i == ch['units'] - 1))
                for ch in ch_by_grp[g]:
                    if ch['last']:
                        finish_block(ch['j'], blk_ps.pop((ch['j'], hf)), hf)

            # ---------------- pooled -> x1 partial
            pooled_sb = smp.tile([cfg.B, cfg.F], f32, tag='pooled')
            for fh in range(2):
                nc.scalar.activation(pooled_sb[:, fh * 512:(fh + 1) * 512],
                                     pool_ps[fh][:], AF.Identity,
                                     scale=float(2.0 ** -8))
            pfm = smp.tile([P, 8, cfg.B], f32, tag='pfm')
            for k in range(8):
                tps = psX.tile([P, cfg.B], f32, space="PSUM", tag='aux')
                nc.tensor.transpose(
                    out=tps[:], in_=pooled_sb[:, k * P:(k + 1) * P],
                    identity=id32[:])
                nc.scalar.activation(pfm[:, k, :], tps[:], AF.Identity)
            xps = psX.tile([P, cfg.B], f32, space="PSUM", tag='aux')
            for k in range(8):
                nc.tensor.matmul(xps[:], lhsT=wpf_sb[:, k, :],
                                 rhs=pfm[:, k, :],
                                 start=(k == 0), stop=(k == 7))
            x1p = smp.tile([P, cfg.B], f32, tag='x1p')
            nc.scalar.activation(x1p[:], xps[:], AF.Identity)

            # ---------------- cc packing + allreduce
            t_x1 = smp.tile([P, cfg.B], f32, tag='tx1')
            t_x2 = smp.tile([P, cfg.B], f32, tag='tx2')
            nc.scalar.activation(t_x1[:], x1p[:], AF.Identity,
                                 scale=mask1_sb[:, 0:1])
            nc.scalar.activation(t_x2[:], x1p[:], AF.Identity,
                                 scale=mask2_sb[:, 0:1])
            bounce_in = drp.tile([CC], f32, tag='ccin')
            bounce_out = drp.tile([N_CORES * CC], f32, tag='ccout')
            seg = P * cfg.B
            for i, t in enumerate((t_x1, t_x2, t_mas)):
                nc.sync.dma_start(
                    out=bounce_in[i * seg:(i + 1) * seg].rearrange(
                        "(p f) -> p f", f=cfg.B),
                    in_=t[:])
            nc.gpsimd.collective_compute(
                "AllGather", OP.bypass,
                replica_groups=[list(range(N_CORES))],
                ins=[bounce_in[:].opt()], outs=[bounce_out[:].opt()])
            gath_v = bounce_out[:].rearrange(
                "(r t p f) -> t p r f", r=N_CORES, t=3, p=P)

            def cc_sum(ti, name):
                raw = smp.tile([P, N_CORES, cfg.B], f32, tag='ccraw',
                               name=f'raw{name}')
                nc.sync.dma_start(out=raw[:], in_=gath_v[ti])
                red = smp.tile([P, cfg.B], f32, tag=f'ccred{name}',
                               name=f'red{name}')
                nc.vector.tensor_reduce(
                    out=red[:], in_=raw[:].transpose([0, 2, 1]),
                    axis=mybir.AxisListType.X, op=OP.add)
                return red

            x12 = {}
            for brr, bpf in ((1, bpf1_sb), (2, bpf2_sb)):
                xs = cc_sum(brr - 1, f'x{brr}')
                nc.scalar.activation(xs[:], xs[:], AF.Lrelu,
                                     bias=bpf[:, 0:1], alpha=0.01)
                x12[brr] = xs
            masif_rb = cc_sum(2, 'mas')

            # ---------------- head
            xc1 = {}
            for mh in range(2):
                cps = psX.tile([P, cfg.B], f32, space="PSUM", tag='aux')
                for k2 in range(2):
                    nc.tensor.matmul(
                        cps[:], lhsT=wfc1_sb[:, k2, mh * P:(mh + 1) * P],
                        rhs=x12[k2 + 1][:], start=(k2 == 0), stop=(k2 == 1))
                xcs = smp.tile([P, cfg.B], f32, tag=f'xc{mh}')
                nc.scalar.activation(xcs[:], cps[:], AF.Lrelu,
                                     bias=bfc1_sb[:, mh, 0:1], alpha=0.01)
                xc1[mh] = xcs
            c2ps = psX.tile([64, cfg.B], f32, space="PSUM", tag='aux')
            for k2 in range(2):
                nc.tensor.matmul(c2ps[:], lhsT=wfc2_sb[:, k2, :],
                                 rhs=xc1[k2][:], start=(k2 == 0),
                                 stop=(k2 == 1))
            xc = smp.tile([64, cfg.B], f32, tag='xcf')
            nc.scalar.activation(xc[:], c2ps[:], AF.Lrelu,
                                 bias=bfc2_sb[:, 0:1], alpha=0.01)

            ops = psX.tile([1, cfg.B], f32, space="PSUM", tag='aux')
            nc.tensor.matmul(ops[:], lhsT=wout_x_sb[:], rhs=xc[:],
                             start=True, stop=False)
            nc.tensor.matmul(ops[:], lhsT=wout_m_sb[:], rhs=masif_rb[:],
                             start=False, stop=True)
            res = smp.tile([1, cfg.B], f32, tag='res')
            nc.scalar.activation(res[:], ops[:], AF.Sigmoid,
                                 bias=bout_sb[:, 0:1])
            nc.sync.dma_start(out=out_t[:, :], in_=res[:])

    nc.compile()
    return nc


# ---------------------------------------------------------------- entry
_CACHE = {}


def _run(inputs, cfg, trace=False, tmpdir=None):
    from concourse import bass_utils
    meta, in_maps = _preprocess(inputs, cfg)
    key = tuple((c['hf'], c['j'], c['units'], c['u0'], c['grp'],
                 c['first'], c['last']) for c in meta['chunks'])
    if key not in _CACHE:
        _CACHE.clear()
        _CACHE[key] = _build(cfg, meta)
    nc = _CACHE[key]
    res = bass_utils.run_bass_kernel_spmd(
        nc, in_maps, core_ids=list(range(N_CORES)), trace=trace, tmpdir=tmpdir)
    out = np.asarray(res.results[0]['out'], np.float32).reshape(cfg.B, 1)
    return out, res


def kernel(**inputs) -> np.ndarray:
    cfg = _Cfg()
    out, _ = _run(inputs, cfg)
    return out


# revision 22
# speedup vs baseline: 1.8393x; 1.0310x over previous
"""Trainium2 Bass kernel for nn_GCNN_87668872446200.

Branch-split design over 8 NeuronCores: cores 0-3 run protein branch 1,
cores 4-7 run branch 2.  Within a branch group each core owns a quarter of
the destination nodes and the full F=1024 feature dim.

Per core (fp8 e4m3 data paths, DoubleRow fp8 matmuls):
  - xw' = 8*(x*dinv_row) @ (W*64) / 8   computed on PE in two source-halves,
    written to HBM as two tensors (xwA rows <5120 + bias row, xwB rest)
  - symmetric norm is separated: h = Dinv A Dinv xw + b realized as
    S-matmul with S[e,d] = dinv[d] (bias via a virtual edge to a bias row)
  - dma_gather pulls 1KB fp8 rows per edge; source-half split lets the
    Q7 descriptor emission of half A overlap the xw compute of half B
  - phase A partial sums staged in SBUF (fp8), injected into phase B PSUM
    via an identity matmul; one ACT pass does lrelu(psum/8)
  - per-graph mean-pool as PE matmul (mpool*256 fp8), W_pf applied locally
  - masif branch (8 graphs/core, this core's branch only)
  - one small AllReduce ([3,128,32] f32 = 48KB) + replicated dense head

All 8 cores run ONE identical program; per-core variation is in input data.
"""
import numpy as np

# ---------------------------------------------------------------- constants
N_CORES = 8
P = 128
BLK = 128           # dest nodes per block
NQ = 4              # dest quarters per branch group
GRPU = 8            # 128-idx units per gather call (1024 idxs)

N_NODES, N_EDGES, F_DIM, B_GRAPHS, L_MAS, C_MAS = 10000, 80000, 1024, 32, 800, 16

USE_DR = True       # DoubleRow fp8 matmuls


class _Cfg:
    def __init__(self, n=N_NODES, e=N_EDGES, f=F_DIM, b=B_GRAPHS,
                 l=L_MAS, c=C_MAS):
        self.N, self.E, self.F, self.B, self.L, self.C = n, e, f, b, l, c
        self.NPAD = ((n + 511) // 512) * 512          # 10240
        self.QH = self.NPAD // NQ                     # 2560 dests per core
        self.NBLK = self.QH // BLK                    # 20 blocks
        self.SH = self.NPAD // 2                      # 5120 source-half split
        self.KC2 = f // 256                           # 4 k-pairs
        self.GPB = b // 4                             # 8 graphs per core
        self.LW = l // 80                             # 10
        self.LB = 8                                   # l-blocks
        self.LBS = l // self.LB                       # 100
        self.WPB = self.LBS // self.LW                # 10
        # xwA holds source rows 0..SH-1 plus bias row (SH) and zero row (SH+1)
        self.XWA_ROWS = self.SH + P                   # 5248
        self.XWB_ROWS = self.NPAD - self.SH           # 5120 (tail rows zero)


def _q8(x):
    import ml_dtypes
    return np.clip(np.asarray(x, np.float32), -240.0, 240.0).astype(
        ml_dtypes.float8_e4m3)


# ---------------------------------------------------------------- host prep
def _edge_plan_core(cfg, edge_index, q):
    """Edges targeting quarter q, split per (block, source-half), sorted.
    Returns dict (j, hf) -> (rows, dests, counts)."""
    row = np.asarray(edge_index[0]).astype(np.int64)
    col = np.asarray(edge_index[1]).astype(np.int64)
    loops = np.arange(cfg.N, dtype=np.int64)
    rows = np.concatenate([row, loops])
    cols = np.concatenate([col, loops])
    lo, hi = q * cfg.QH, (q + 1) * cfg.QH
    sel = (cols >= lo) & (cols < hi)
    r, c = rows[sel], cols[sel] - lo
    out = {}
    for j in range(cfg.NBLK):
        bsel = (c >= j * BLK) & (c < (j + 1) * BLK)
        rj, cj = r[bsel], c[bsel] - j * BLK
        for hf in range(2):
            hsel = (rj < cfg.SH) if hf == 0 else (rj >= cfg.SH)
            out[(j, hf)] = (rj[hsel], cj[hsel])
    return out


def _shared_schedule(cfg, plans):
    """Shared chunk schedule (max over the 8 per-core plans).

    Returns chunks: list of dicts with keys
      hf, j, units (1 or 2), u0 (unit offset in group), grp (group index),
      first (starts block), last (ends block's half... block completion is
      tracked at (j,hf==1,last) for B and (j,hf==0,last) for A)
    and n_groups_a / n_groups_b.
    """
    # per-plan unit needs and slot permutation (sort blocks big->small so the
    # max-over-cores slot schedule aligns; mpool/idx/smat are per-core data)
    def units_of(p, j, hf):
        e = len(p[(j, hf)][0]) + (1 if hf == 0 else 0)
        return max(1, (e + P - 1) // P)

    perms = []
    for p in plans:
        tot = [units_of(p, j, 0) + units_of(p, j, 1) for j in range(cfg.NBLK)]
        perms.append(list(np.argsort(-np.asarray(tot), kind='stable')))

    slot_need = {}
    for s in range(cfg.NBLK):
        for hf in range(2):
            slot_need[(s, hf)] = max(
                units_of(p, perm[s], hf) for p, perm in zip(plans, perms))

    chunks = []
    groups = []                                   # list of [hf, nunits]
    for hf in range(2):
        space = 0                                 # force new group per half
        for s in range(cfg.NBLK):
            left = slot_need[(s, hf)]
            first = True
            while left:
                if space == 0:
                    groups.append([hf, 0])
                    space = GRPU
                sz = 2 if (left >= 2 and space >= 2) else 1
                chunks.append(dict(hf=hf, j=s, units=sz, u0=GRPU - space,
                                   grp=len(groups) - 1,
                                   first=first, last=(left - sz == 0)))
                space -= sz
                left -= sz
                first = False
                groups[-1][1] = GRPU - space
    return chunks, groups, perms


def _fill_core_gather2(cfg, chunks, groups, plan, dinv, q, perm):
    """Per-core idx + smat content for the shared schedule (slot j maps to
    physical block perm[j] for this core)."""
    n_groups = len(groups)
    flat_idx = np.zeros((n_groups, GRPU * P), np.int64)
    smat = np.zeros((n_groups, P, GRPU * P), np.float32)
    dinv8 = _q8(dinv).astype(np.float32)
    consumed = {}
    for ch in chunks:
        sl, hf, g, u0 = ch['j'], ch['hf'], ch['grp'], ch['u0']
        j = perm[sl]
        r, c = plan[(j, hf)]
        off = consumed.get((j, hf), 0)
        cap = ch['units'] * P
        base = u0 * P
        pad_idx = cfg.SH + 1 if hf == 0 else cfg.XWB_ROWS - 1
        flat_idx[g, base:base + cap] = pad_idx
        s = 0
        if hf == 0 and ch['first']:
            flat_idx[g, base] = cfg.SH            # bias row at slot 0
            smat[g, 0, base:base + P] = 1.0
            s = 1
        take = min(len(r) - off, cap - s)
        if take > 0:
            rr = r[off:off + take]
            cc = c[off:off + take]
            if hf == 1:
                rr = rr - cfg.SH
            slots = np.arange(s, s + take) + base
            up = slots // P
            pp = slots % P
            flat_idx[g, slots] = rr
            # dest scale dinv[global dest] ; global dest = q*QH + j*BLK + cc
            gd = q * cfg.QH + j * BLK + cc
            vals = dinv8[np.minimum(gd, cfg.N - 1)] * (gd < cfg.N)
            smat[g, pp, up * P + cc] = vals
        consumed[(j, hf)] = off + take
    for (j, hf), off in consumed.items():
        assert off == len(plan[(j, hf)][0]), (j, hf, off, len(plan[(j, hf)][0]))
    return flat_idx, smat


def _wrap_idx_groups(flat_idx):
    """[G, 1024] -> [128, G*64] int16 (16-part wrap, 8x replicated)."""
    g, n = flat_idx.shape
    w = flat_idx.reshape(g, n // 16, 16).transpose(2, 0, 1).reshape(16, -1)
    return np.tile(w, (8, 1)).astype(np.int16)


def _preprocess(inputs, cfg):
    import ml_dtypes
    bf16 = ml_dtypes.bfloat16
    f32 = np.float32

    # --- per-branch shared data
    bdata = {}
    for br in (1, 2):
        x = np.asarray(inputs[f'pro{br}_x'], f32)
        ei = np.asarray(inputs[f'pro{br}_edge_index'])
        batch = np.asarray(inputs[f'pro{br}_batch']).astype(np.int64)
        row = ei[0].astype(np.int64)
        col = ei[1].astype(np.int64)
        deg = np.bincount(np.concatenate([col, np.arange(cfg.N)]),
                          minlength=cfg.N).astype(np.float64)
        dinv = (1.0 / np.sqrt(deg)).astype(f32)
        # xt_dr [NSLAB, 128, KC2*2*512] fp8 of (x*dinv_row)^T, slab-major
        xp = x * dinv[:, None]
        xpT = np.zeros((cfg.F, cfg.NPAD), f32)
        xpT[:, :cfg.N] = xp.T
        nslab = cfg.NPAD // 512
        xt_dr = np.ascontiguousarray(
            xpT.reshape(cfg.KC2, 2, P, nslab, 512).transpose(3, 2, 0, 1, 4)
        ).reshape(nslab, P, cfg.KC2 * 2 * 512)
        # wg_dr [128, KC2*2*F] fp8  (p, c, i, f)
        W = np.asarray(inputs[f'W_g{br}'], f32) * 64.0
        wg_dr = np.ascontiguousarray(
            W.reshape(cfg.KC2, 2, P, cfg.F).transpose(2, 0, 1, 3)
        ).reshape(P, cfg.KC2 * 2 * cfg.F)
        b8 = np.asarray(inputs[f'b_g{br}'], f32) * 8.0
        cnt = np.bincount(batch, minlength=cfg.B).astype(f32)
        plans = [_edge_plan_core(cfg, ei, q) for q in range(NQ)]
        bdata[br] = dict(xt=_q8(xt_dr), wg=_q8(wg_dr), b8=_q8(b8[None, :]),
                         dinv=dinv, batch=batch, cnt=cnt, plans=plans)

    # --- shared chunk schedule (max over all 8 core plans, slot-permuted)
    all_plans = bdata[1]['plans'] + bdata[2]['plans']
    chunks, groups, perms = _shared_schedule(cfg, all_plans)
    n_groups = len(groups)
    n_ga = sum(1 for g in groups if g[0] == 0)

    meta = dict(chunks=chunks, groups=groups, n_groups=n_groups, n_ga=n_ga)

    # --- head weights (shared across cores)
    shared = {}
    shared['wfc1'] = np.ascontiguousarray(
        np.asarray(inputs['W_fc1'], f32).reshape(2, P, 256).transpose(1, 0, 2))
    shared['wfc2'] = np.ascontiguousarray(
        np.asarray(inputs['W_fc2'], f32).reshape(2, P, 64).transpose(1, 0, 2))
    shared['bfc1'] = np.ascontiguousarray(
        np.asarray(inputs['b_fc1'], f32).reshape(2, P, 1).transpose(1, 0, 2))
    shared['bfc2'] = np.asarray(inputs['b_fc2'], f32).reshape(64, 1)
    wout = np.asarray(inputs['W_out'], f32)
    shared['wout_x'] = np.ascontiguousarray(wout[0:64])            # [64,1]
    shared['wout_m'] = np.ascontiguousarray(wout[64:192])          # [128,1]
    shared['bout'] = np.asarray(inputs['b_out'], f32).reshape(1, 1)
    shared['bpf1'] = np.asarray(inputs['b_pf1'], f32).reshape(P, 1)
    shared['bpf2'] = np.asarray(inputs['b_pf2'], f32).reshape(P, 1)
    shared['id32'] = np.eye(32, dtype=f32)
    shared['id128_8'] = _q8(np.eye(P, dtype=f32))
    shared['id64'] = np.eye(64, dtype=f32)

    in_maps = []
    for core in range(N_CORES):
        br = 1 + core // NQ
        q = core % NQ
        bd = bdata[br]
        m = dict(shared)
        m['xt'] = bd['xt']
        m['wg'] = bd['wg']
        m['b8row'] = bd['b8']
        # gather plan
        perm = perms[core]
        flat_idx, smat = _fill_core_gather2(
            cfg, chunks, groups, bd['plans'][q], bd['dinv'], q, perm)
        m['idx'] = _wrap_idx_groups(flat_idx)
        m['smat'] = np.ascontiguousarray(
            smat.transpose(1, 0, 2).reshape(P, n_groups * GRPU * P)).astype(
            ml_dtypes.float8_e4m3)
        # mpool [128, NBLK, B] fp8 (x256); slot j -> physical block perm[j]
        mp = np.zeros((P, cfg.NBLK, cfg.B), f32)
        for j in range(cfg.NBLK):
            nodes = q * cfg.QH + perm[j] * BLK + np.arange(BLK)
            ok = nodes < cfg.N
            gidx = bd['batch'][np.minimum(nodes, cfg.N - 1)]
            val = 256.0 / np.maximum(bd['cnt'][gidx], 1.0) * ok
            mp[np.arange(BLK), j, gidx] = val
        m['mpool'] = _q8(mp.reshape(P, cfg.NBLK * cfg.B))
        # W_pf for this branch  [128, 8*128] f32  (p, k, m)
        wpf = np.asarray(inputs[f'W_pf{br}'], f32)
        m['wpf'] = np.ascontiguousarray(
            wpf.reshape(8, P, P).transpose(1, 0, 2)).reshape(P, 8 * P)
        # branch masks for cc packing
        m['mask1'] = np.full((P, 1), 1.0 if br == 1 else 0.0, f32)
        m['mask2'] = np.full((P, 1), 1.0 if br == 2 else 0.0, f32)
        # masif (this branch only, 8 graphs) laid out [64=(lb,g), C*LBS]
        gs = (core % NQ) * cfg.GPB
        for sfk, name in (('s', 'straight'), ('f', 'flipped')):
            src = np.asarray(inputs[f'mas{br}_{name}'], f32)[gs:gs + cfg.GPB]
            # [g, ch, lb*LBS+l] -> [(lb, g), ch, l]
            r = src.reshape(cfg.GPB, cfg.C, cfg.LB, cfg.LBS).transpose(
                2, 0, 1, 3)
            m[f'mas_{sfk}'] = np.ascontiguousarray(r).reshape(
                64, cfg.C * cfg.LBS)
        # wm128 [10, 8, 128]: this branch's W_m/(2*LW) at rows 64*(br-1)..
        wm = np.zeros((cfg.WPB, cfg.LB, P), f32)
        wsrc = (np.asarray(inputs[f'W_m{br}'], f32) / (2.0 * cfg.LW)).reshape(
            cfg.LB, cfg.WPB, 64)
        wm[:, :, 64 * (br - 1):64 * br] = wsrc.transpose(1, 0, 2)
        m['wm'] = np.ascontiguousarray(wm)
        bm = np.zeros((P, 1), f32)
        bm[64 * (br - 1):64 * br, 0] = np.asarray(inputs[f'b_m{br}'], f32)
        m['bm'] = bm
        gm = np.zeros((P, cfg.B), f32)
        gm[64 * (br - 1):64 * br, gs:gs + cfg.GPB] = 1.0
        m['gmask'] = gm
        for sf, pre in (('s', 'cs'), ('f', 'cf')):
            w = float(np.asarray(inputs[f'{pre}{br}_w'])[0])
            b = float(np.asarray(inputs[f'{pre}{br}_b'])[0])
            m[f'scale_{sf}'] = np.full((64, 1), w / cfg.C, f32)
            m[f'bias_{sf}'] = np.full((64, 1), b, f32)
        in_maps.append(m)
    return meta, in_maps


# ---------------------------------------------------------------- program
def _build(cfg, meta):
    import concourse.bass as bass
    import concourse.bacc as bacc
    import concourse.mybir as mybir
    import concourse.tile as tile

    dt = mybir.dt
    f32 = dt.float32
    fp8 = dt.float8e4
    AF = mybir.ActivationFunctionType
    OP = mybir.AluOpType
    DR = mybir.MatmulPerfMode.DoubleRow if USE_DR else None

    chunks = meta['chunks']
    groups = meta['groups']
    n_groups = meta['n_groups']
    n_ga = meta['n_ga']

    nc = bacc.Bacc("TRN2", target_bir_lowering=False, debug=False,
                   enable_asserts=False, num_devices=N_CORES)

    def din(name, shape, d):
        return nc.dram_tensor(name, list(shape), d, kind="ExternalInput")

    NSLAB = cfg.NPAD // 512
    xt_d = din('xt', (NSLAB, P, cfg.KC2 * 2 * 512), fp8)
    wg_d = din('wg', (P, cfg.KC2 * 2 * cfg.F), fp8)
    b8_d = din('b8row', (1, cfg.F), fp8)
    idx_d = din('idx', (P, n_groups * 64), dt.int16)
    smat_d = din('smat', (P, n_groups * GRPU * P), fp8)
    mpool_d = din('mpool', (P, cfg.NBLK * cfg.B), fp8)
    wpf_d = din('wpf', (P, 8 * P), f32)
    mas_d = {sf: din(f'mas_{sf}', (64, cfg.C * cfg.LBS), f32) for sf in 'sf'}
    wm_d = din('wm', (cfg.WPB, cfg.LB, P), f32)
    bm_d = din('bm', (P, 1), f32)
    gmask_d = din('gmask', (P, cfg.B), f32)
    msc_d = {(sf, kind): din(f'{kind}_{sf}', (64, 1), f32)
             for sf in 'sf' for kind in ('scale', 'bias')}
    wfc1_d = din('wfc1', (P, 2, 256), f32)
    wfc2_d = din('wfc2', (P, 2, 64), f32)
    bfc1_d = din('bfc1', (P, 2, 1), f32)
    bfc2_d = din('bfc2', (64, 1), f32)
    wout_x_d = din('wout_x', (64, 1), f32)
    wout_m_d = din('wout_m', (P, 1), f32)
    bout_d = din('bout', (1, 1), f32)
    bpf1_d = din('bpf1', (P, 1), f32)
    bpf2_d = din('bpf2', (P, 1), f32)
    mask1_d = din('mask1', (P, 1), f32)
    mask2_d = din('mask2', (P, 1), f32)
    id32_d = din('id32', (32, 32), f32)
    id64_d = din('id64', (64, 64), f32)
    id128_d = din('id128_8', (P, P), fp8)

    out_t = nc.dram_tensor('out', [1, cfg.B], f32, kind="ExternalOutput")

    CC = 3 * P * cfg.B          # allreduce payload (f32 elements)

    with tile.TileContext(nc) as tc:
        with tc.tile_pool(name="const", bufs=1) as cst, \
             tc.tile_pool(name="xt", bufs=2) as xtp, \
             tc.tile_pool(name="xwsb", bufs=3) as xwsb, \
             tc.tile_pool(name="gat", bufs=5) as gatp, \
             tc.tile_pool(name="hsb", bufs=2) as hp, \
             tc.tile_pool(name="small", bufs=2) as smp, \
             tc.tile_pool(name="psA", bufs=2, space="PSUM") as psA, \
             tc.tile_pool(name="psBlk", bufs=1, space="PSUM") as psB, \
             tc.tile_pool(name="psPool", bufs=1, space="PSUM") as psP, \
             tc.tile_pool(name="psX", bufs=2, space="PSUM") as psX, \
             tc.tile_pool(name="dram", bufs=1, space="DRAM") as drp:

            def load(pool, src_ap, shape, d, name=None):
                t = pool.tile(list(shape), d, tag=name)
                nc.sync.dma_start(out=t[:], in_=src_ap)
                return t

            # ---------------- xw-critical constants first
            wg_flat = load(cst, wg_d[:, :], (P, cfg.KC2 * 2 * cfg.F), fp8,
                           'wg')
            wg_sb = wg_flat[:].rearrange("p (c i f) -> p c i f", c=cfg.KC2,
                                         i=2)

            # ---------------- xw compute: A half (source rows < SH)
            xwA = drp.tile([cfg.XWA_ROWS, cfg.F], fp8, tag='xwA')
            xwB = drp.tile([cfg.XWB_ROWS, cfg.F], fp8, tag='xwB')

            def xw_slab(sl):
                n0 = sl * 512
                xt_flat = xtp.tile([P, cfg.KC2 * 2 * 512], fp8, tag='xt',
                                   name='xt_t')
                nc.sync.dma_start(out=xt_flat[:], in_=xt_d[sl, :, :])
                xt_t = xt_flat[:].rearrange("p (c i n) -> p c i n", c=cfg.KC2,
                                            i=2)
                for sub in range(4):
                    xw_t = xwsb.tile([P, cfg.F], fp8, tag='xwsb', name='xw_t')
                    for fh in range(2):
                        ps = psA.tile([P, 512], f32, space="PSUM", tag='xwps',
                                      name='xw_ps')
                        for c in range(cfg.KC2):
                            if USE_DR:
                                nc.tensor.matmul(
                                    ps[:],
                                    lhsT=xt_t[:, c, :, sub * P:(sub + 1) * P],
                                    rhs=wg_sb[:, c, :, fh * 512:(fh + 1) * 512],
                                    start=(c == 0), stop=(c == cfg.KC2 - 1),
                                    perf_mode=DR)
                            else:
                                for i in range(2):
                                    nc.tensor.matmul(
                                        ps[:],
                                        lhsT=xt_t[:, c, i,
                                                  sub * P:(sub + 1) * P],
                                        rhs=wg_sb[:, c, i,
                                                  fh * 512:(fh + 1) * 512],
                                        start=(c == 0 and i == 0),
                                        stop=(c == cfg.KC2 - 1 and i == 1))
                        nc.scalar.activation(xw_t[:, fh * 512:(fh + 1) * 512],
                                             ps[:], AF.Identity, scale=0.125)
                    row = n0 + sub * P
                    if row < cfg.SH:
                        nc.sync.dma_start(out=xwA[row:row + P, :], in_=xw_t[:])
                    else:
                        nc.sync.dma_start(
                            out=xwB[row - cfg.SH:row - cfg.SH + P, :],
                            in_=xw_t[:])

            for sl in range(NSLAB // 2):
                xw_slab(sl)

            # ---------------- remaining constants (overlap with xw PE)
            idx_sb = load(cst, idx_d[:, :], (P, n_groups * 64), dt.int16, 'idx')
            smat_sb = load(cst, smat_d[:, :], (P, n_groups * GRPU * P), fp8,
                           'smat')
            mpool_sb = load(cst, mpool_d[:, :], (P, cfg.NBLK * cfg.B), fp8,
                            'mpool')
            wpf_flat = load(cst, wpf_d[:, :], (P, 8 * P), f32, 'wpf')
            wpf_sb = wpf_flat[:].rearrange("p (k m) -> p k m", k=8)
            wm_sb = load(cst, wm_d[:, :, :], (cfg.WPB, cfg.LB, P), f32, 'wm')
            bm_sb = load(cst, bm_d[:, :], (P, 1), f32, 'bm')
            gmask_sb = load(cst, gmask_d[:, :], (P, cfg.B), f32, 'gmask')
            msc_sb = {k: load(cst, v[:, :], (64, 1), f32, f'msc{k}')
                      for k, v in msc_d.items()}
            wfc1_sb = load(cst, wfc1_d[:, :, :], (P, 2, 256), f32, 'wfc1')
            wfc2_sb = load(cst, wfc2_d[:, :, :], (P, 2, 64), f32, 'wfc2')
            bfc1_sb = load(cst, bfc1_d[:, :, :], (P, 2, 1), f32, 'bfc1')
            bfc2_sb = load(cst, bfc2_d[:, :], (64, 1), f32, 'bfc2')
            wout_x_sb = load(cst, wout_x_d[:, :], (64, 1), f32, 'woutx')
            wout_m_sb = load(cst, wout_m_d[:, :], (P, 1), f32, 'woutm')
            bout_sb = load(cst, bout_d[:, :], (1, 1), f32, 'bout')
            bpf1_sb = load(cst, bpf1_d[:, :], (P, 1), f32, 'bpf1')
            bpf2_sb = load(cst, bpf2_d[:, :], (P, 1), f32, 'bpf2')
            mask1_sb = load(cst, mask1_d[:, :], (P, 1), f32, 'mask1')
            mask2_sb = load(cst, mask2_d[:, :], (P, 1), f32, 'mask2')
            id32 = load(cst, id32_d[:, :], (32, 32), f32, 'id32')
            id64 = load(cst, id64_d[:, :], (64, 64), f32, 'id64')
            id128 = load(cst, id128_d[:, :], (P, P), fp8, 'id128')
            b8_sb = load(cst, b8_d[:, :], (1, cfg.F), fp8, 'b8')

            hA = cst.tile([P, cfg.NBLK * cfg.F], fp8, tag='hA')

            # ---------------- masif (one branch, 8 graphs -> [128, B] via PE)
            frag = None
            for sf in 'sf':
                tf = smp.tile([64, cfg.C * cfg.LBS], f32, tag='masload',
                              name='mas_t')
                nc.sync.dma_start(out=tf[:], in_=mas_d[sf][:, :])
                t = tf[:].rearrange("p (c l) -> p c l", c=cfg.C)
                red = smp.tile([64, cfg.LBS], f32, tag='masred')
                nc.vector.tensor_reduce(
                    out=red[:], in_=t.transpose([0, 2, 1]),
                    axis=mybir.AxisListType.X, op=OP.add)
                act = smp.tile([64, cfg.LBS], f32, tag='masact')
                nc.scalar.activation(
                    act[:], red[:], AF.Relu,
                    bias=msc_sb[(sf, 'bias')][:, 0:1],
                    scale=msc_sb[(sf, 'scale')][:, 0:1])
                ws = smp.tile([64, cfg.WPB], f32, tag='masws')
                nc.vector.tensor_reduce(
                    out=ws[:],
                    in_=act[:].rearrange("p (w l) -> p w l", l=cfg.LW),
                    axis=mybir.AxisListType.X, op=OP.add)
                if frag is None:
                    frag = ws
                else:
                    frag2 = smp.tile([64, cfg.WPB], f32, tag='masfrag')
                    nc.vector.tensor_add(out=frag2[:], in0=frag[:], in1=ws[:])
                    frag = frag2
            ps_t = psX.tile([cfg.WPB, 64], f32, space="PSUM", tag='aux')
            nc.tensor.transpose(out=ps_t[:], in_=frag[:], identity=id64[:])
            fragT = smp.tile([cfg.WPB, 64], f32, tag='masfragT')
            nc.scalar.activation(fragT[:], ps_t[:], AF.Identity)
            m_ps = psX.tile([P, cfg.GPB], f32, space="PSUM", tag='aux')
            for lb in range(cfg.LB):
                nc.tensor.matmul(
                    m_ps[:], lhsT=wm_sb[:, lb, :],
                    rhs=fragT[:, lb * cfg.GPB:(lb + 1) * cfg.GPB],
                    start=(lb == 0), stop=(lb == cfg.LB - 1))
            m_fm = smp.tile([P, cfg.GPB], f32, tag='masfm')
            nc.scalar.activation(m_fm[:], m_ps[:], AF.Identity,
                                 bias=bm_sb[:, 0:1])
            t_mas = cst.tile([P, cfg.B], f32, tag='tmas')
            nc.vector.tensor_tensor(
                out=t_mas[:].rearrange("p (s g) -> p s g", g=cfg.GPB),
                in0=m_fm[:, None, :].to_broadcast([P, NQ, cfg.GPB]),
                in1=gmask_sb[:, :].rearrange("p (s g) -> p s g", g=cfg.GPB),
                op=OP.mult)

            # bias row + zero row of xwA, then B-half slabs
            zrow = smp.tile([1, cfg.F], fp8, tag='zrow')
            nc.vector.memset(zrow[:], 0.0)
            nc.sync.dma_start(out=xwA[cfg.SH:cfg.SH + 1, :], in_=b8_sb[:])
            nc.sync.dma_start(out=xwA[cfg.SH + 1:cfg.SH + 2, :], in_=zrow[:])

            for sl in range(NSLAB // 2, NSLAB):
                xw_slab(sl)

            # ---------------- gather + scatter + pool
            pool_ps = [psP.tile([cfg.B, 512], f32, space="PSUM",
                                name=f'poolps{fh}') for fh in range(2)]
            blk_ps = {}
            gat_tiles = {}
            # per-group gathers; chunks reference their group's tile
            ch_by_grp = {}
            for ch in chunks:
                ch_by_grp.setdefault(ch['grp'], []).append(ch)

            pooled_n = [0]

            def finish_block(j, ps_pair, phase):
                if phase == 0:
                    # stage A partial (8x scale) into hA as fp8
                    for fh in range(2):
                        nc.scalar.activation(
                            hA[:, j * cfg.F + fh * 512:
                               j * cfg.F + (fh + 1) * 512],
                            ps_pair[fh][:], AF.Identity)
                else:
                    h_t = hp.tile([P, cfg.F], fp8, tag='h')
                    for fh in range(2):
                        nc.scalar.activation(
                            h_t[:, fh * 512:(fh + 1) * 512], ps_pair[fh][:],
                            AF.Lrelu, scale=0.125, alpha=0.01)
                    for fh in range(2):
                        nc.tensor.matmul(
                            pool_ps[fh][:],
                            lhsT=mpool_sb[:, j * cfg.B:(j + 1) * cfg.B],
                            rhs=h_t[:, fh * 512:(fh + 1) * 512],
                            start=(pooled_n[0] == 0),
                            stop=(pooled_n[0] == cfg.NBLK - 1))
                    pooled_n[0] += 1

            for g in range(n_groups):
                hf = groups[g][0]
                src = xwA if hf == 0 else xwB
                gat = gatp.tile([P, GRPU, cfg.F], fp8, tag='gat')
                nc.gpsimd.dma_gather(
                    out_ap=gat[:], in_ap=src[:, :],
                    idxs_ap=idx_sb[:, g * 64:(g + 1) * 64],
                    num_idxs=GRPU * P, num_idxs_reg=GRPU * P,
                    elem_size=cfg.F)
                for ch in ch_by_grp[g]:
                    j, u0 = ch['j'], ch['u0']
                    if ch['first']:
                        pair = [psB.tile([P, 512], f32, space="PSUM",
                                         name=f'blkps{fh}') for fh in range(2)]
                        blk_ps[(j, hf)] = pair
                        if hf == 1:
                            for fh in range(2):
                                nc.tensor.matmul(
                                    pair[fh][:], lhsT=id128[:],
                                    rhs=hA[:, j * cfg.F + fh * 512:
                                           j * cfg.F + (fh + 1) * 512],
                                    start=True, stop=False)
                    pair = blk_ps[(j, hf)]
                    sm0 = (g * GRPU + u0) * P
                    st = ch['first'] and hf == 0
                    sp = ch['last']
                    for fh in range(2):
                        if ch['units'] == 2 and USE_DR:
                            nc.tensor.matmul(
                                pair[fh][:],
                                lhsT=smat_sb[:, sm0:sm0 + 2 * P].rearrange(
                                    "p (i d) -> p i d", i=2),
                                rhs=gat[:, u0:u0 + 2,
                                        fh * 512:(fh + 1) * 512],
                                start=st, stop=sp, perf_mode=DR)
                        else:
                            for i in range(ch['units']):
                                nc.tensor.matmul(
                                    pair[fh][:],
                                    lhsT=smat_sb[:, sm0 + i * P:
                                                 sm0 + (i + 1) * P],
                                    rhs=gat[:, u0 + i,
                                            fh * 512:(fh + 1) * 512],
                                    start=(st and i == 0),
                                    stop=(sp and i == ch['units'] - 1))
                for ch in ch_by_grp[g]:
                    if ch['last']:
                        finish_block(ch['j'], blk_ps.pop((ch['j'], hf)), hf)

            # ---------------- pooled -> x1 partial
            pooled_sb = smp.tile([cfg.B, cfg.F], f32, tag='pooled')
            for fh in range(2):
                nc.scalar.activation(pooled_sb[:, fh * 512:(fh + 1) * 512],
                                     pool_ps[fh][:], AF.Identity,
                                     scale=float(2.0 ** -8))
            pfm = smp.tile([P, 8, cfg.B], f32, tag='pfm')
            for k in range(8):
                tps = psX.tile([P, cfg.B], f32, space="PSUM", tag='aux')
                nc.tensor.transpose(
                    out=tps[:], in_=pooled_sb[:, k * P:(k + 1) * P],
                    identity=id32[:])
                nc.scalar.activation(pfm[:, k, :], tps[:], AF.Identity)
            xps = psX.tile([P, cfg.B], f32, space="PSUM", tag='aux')
            for k in range(8):
                nc.tensor.matmul(xps[:], lhsT=wpf_sb[:, k, :],
                                 rhs=pfm[:, k, :],
                                 start=(k == 0), stop=(k == 7))
            x1p = smp.tile([P, cfg.B], f32, tag='x1p')
            nc.scalar.activation(x1p[:], xps[:], AF.Identity)

            # ---------------- cc packing + allreduce
            t_x1 = smp.tile([P, cfg.B], f32, tag='tx1')
            t_x2 = smp.tile([P, cfg.B], f32, tag='tx2')
            nc.scalar.activation(t_x1[:], x1p[:], AF.Identity,
                                 scale=mask1_sb[:, 0:1])
            nc.scalar.activation(t_x2[:], x1p[:], AF.Identity,
                                 scale=mask2_sb[:, 0:1])
            bounce_in = drp.tile([CC], f32, tag='ccin')
            bounce_out = drp.tile([N_CORES * CC], f32, tag='ccout')
            seg = P * cfg.B
            for i, t in enumerate((t_x1, t_x2, t_mas)):
                nc.sync.dma_start(
                    out=bounce_in[i * seg:(i + 1) * seg].rearrange(
                        "(p f) -> p f", f=cfg.B),
                    in_=t[:])
            nc.gpsimd.collective_compute(
                "AllGather", OP.bypass,
                replica_groups=[list(range(N_CORES))],
                ins=[bounce_in[:].opt()], outs=[bounce_out[:].opt()])
            gath_v = bounce_out[:].rearrange(
                "(r t p f) -> t p r f", r=N_CORES, t=3, p=P)

            def cc_sum(ti, name):
                raw = smp.tile([P, N_CORES, cfg.B], f32, tag='ccraw',
                               name=f'raw{name}')
                nc.sync.dma_start(out=raw[:], in_=gath_v[ti])
                red = smp.tile([P, cfg.B], f32, tag=f'ccred{name}',
                               name=f'red{name}')
                nc.vector.tensor_reduce(
                    out=red[:], in_=raw[:].transpose([0, 2, 1]),
                    axis=mybir.AxisListType.X, op=OP.add)
                return red

            x12 = {}
            for brr, bpf in ((1, bpf1_sb), (2, bpf2_sb)):
                xs = cc_sum(brr - 1, f'x{brr}')
                nc.scalar.activation(xs[:], xs[:], AF.Lrelu,
                                     bias=bpf[:, 0:1], alpha=0.01)
                x12[brr] = xs
            masif_rb = cc_sum(2, 'mas')

            # ---------------- head
            xc1 = {}
            for mh in range(2):
                cps = psX.tile([P, cfg.B], f32, space="PSUM", tag='aux')
                for k2 in range(2):
                    nc.tensor.matmul(
                        cps[:], lhsT=wfc1_sb[:, k2, mh * P:(mh + 1) * P],
                        rhs=x12[k2 + 1][:], start=(k2 == 0), stop=(k2 == 1))
                xcs = smp.tile([P, cfg.B], f32, tag=f'xc{mh}')
                nc.scalar.activation(xcs[:], cps[:], AF.Lrelu,
                                     bias=bfc1_sb[:, mh, 0:1], alpha=0.01)
                xc1[mh] = xcs
            c2ps = psX.tile([64, cfg.B], f32, space="PSUM", tag='aux')
            for k2 in range(2):
                nc.tensor.matmul(c2ps[:], lhsT=wfc2_sb[:, k2, :],
                                 rhs=xc1[k2][:], start=(k2 == 0),
                                 stop=(k2 == 1))
            xc = smp.tile([64, cfg.B], f32, tag='xcf')
            nc.scalar.activation(xc[:], c2ps[:], AF.Lrelu,
                                 bias=bfc2_sb[:, 0:1], alpha=0.01)

            ops = psX.tile([1, cfg.B], f32, space="PSUM", tag='aux')
            nc.tensor.matmul(ops[:], lhsT=wout_x_sb[:], rhs=xc[:],
                             start=True, stop=False)
            nc.tensor.matmul(ops[:], lhsT=wout_m_sb[:], rhs=masif_rb[:],
                             start=False, stop=True)
            res = smp.tile([1, cfg.B], f32, tag='res')
            nc.scalar.activation(res[:], ops[:], AF.Sigmoid,
                                 bias=bout_sb[:, 0:1])
            nc.sync.dma_start(out=out_t[:, :], in_=res[:])

    nc.compile()
    return nc


# ---------------------------------------------------------------- entry
_CACHE = {}


def _run(inputs, cfg, trace=False, tmpdir=None):
    from concourse import bass_utils
    meta, in_maps = _preprocess(inputs, cfg)
    key = tuple((c['hf'], c['j'], c['units'], c['u0'], c['grp'],
                 c['first'], c['last']) for c in meta['chunks'])
    if key not in _CACHE:
        _CACHE.clear()
        _CACHE[key] = _build(cfg, meta)
    nc = _CACHE[key]
    res = bass_utils.run_bass_kernel_spmd(
        nc, in_maps, core_ids=list(range(N_CORES)), trace=trace, tmpdir=tmpdir)
    out = np.asarray(res.results[0]['out'], np.float32).reshape(cfg.B, 1)
    return out, res


def kernel(**inputs) -> np.ndarray:
    cfg = _Cfg()
    out, _ = _run(inputs, cfg)
    return out
